# revision 1
# baseline (speedup 1.0000x reference)
"""Multi-head attention block on 8 Trainium2 NeuronCores.

Problem: B=8, N=1024, E=768, H=12, D=64 attention (QKV proj -> softmax(QK^T/8)V
-> output proj), fp32 I/O.

Sharding: data parallel over batch — core b computes batch element b entirely
locally; no collectives. Host shards/stacks.

Per-core kernel (matmuls in fp32r — hardware TF32-like mode, 1 cyc/row):
  phase 0: DMA x -> PE-transpose (batched 4 per psum tile) -> xT [E, N]
  phase 1: V natural [N, 65*12] with a ones column per head (col 65h+64),
           then qT/kT pairs [128, N]: rows (h%2)*64 hold head h's d-dims.
           Q/K bias via per-partition scalar add at psum evac; V/proj bias
           via ones-outer-product broadcast tiles added at evac.
  phase 2: per head: S^T[k,q] = K^T.T Q^T (two 512-wide matmuls into one
           [128,1024] psum); exp on ACT (scale=1/8, one 1024-wide op);
           U_aug[65, q] = [V | 1].T expS (row 64 = softmax denominator Z);
           invZ = 1/Z (f32r); K=1 matmul broadcasts invZ to 64 rows;
           attnT pair tile rows 0:64 (even head, DVE) / 64:128 (odd head,
           DVE -> staging -> partition-shift SBUF DMA)
  phase 3: out[t, e] = sum_c attnT[c].T W_proj[c] + b_proj (128-contraction)
"""
import numpy as np

B, N, E, H, D = 8, 1024, 768, 12, 64
SCALE = D ** -0.5
NT = N // 128   # token chunks (8)
NE = E // 128   # embed chunks (6)
NQ = N // 512   # moving-dim tiles (2)
NFS = [(0, 512), (512, 256)]  # free-dim split of E for matmuls


def _build():
    import concourse.bacc as bacc
    import concourse.mybir as mybir
    import concourse.tile as tile
    from concourse.masks import make_identity

    F32 = mybir.dt.float32
    F32R = mybir.dt.float32r
    EXP = mybir.ActivationFunctionType.Exp

    nc = bacc.Bacc("TRN2", target_bir_lowering=False)
    x_d = nc.declare_dram_parameter("x", [N, E], F32, isOutput=False)
    wqkv_d = nc.declare_dram_parameter("W_qkv", [E, 3 * E], F32, isOutput=False)
    bqkv_d = nc.declare_dram_parameter("b_qkv", [3 * E], F32, isOutput=False)
    wproj_d = nc.declare_dram_parameter("W_proj", [E, E], F32, isOutput=False)
    bproj_d = nc.declare_dram_parameter("b_proj", [E], F32, isOutput=False)
    out_d = nc.declare_dram_parameter("out", [N, E], F32, isOutput=True)

    with tile.TileContext(nc) as tc:
        with (
            tc.tile_pool(name="const", bufs=1) as cp,
            tc.tile_pool(name="qkv", bufs=1) as qp,
            tc.tile_pool(name="psum", bufs=1, space="PSUM") as ps,
        ):
            # ---- constants ----
            identf = cp.tile([128, 128], F32)
            make_identity(nc, identf)
            ident = cp.tile([128, 128], F32R)
            nc.vector.tensor_copy(ident, identf)
            ones1f = cp.tile([1, 128], F32)
            nc.vector.memset(ones1f, 1.0)
            ones1 = cp.tile([1, 128], F32R)
            nc.vector.tensor_copy(ones1, ones1f)
            ones65f = cp.tile([65, 64], F32)
            nc.vector.memset(ones65f, 1.0)
            ones65 = cp.tile([65, 64], F32R)
            nc.vector.tensor_copy(ones65, ones65f)
            bq_cols = [cp.tile([128, 1], F32, name=f"bq_{fc}", tag=f"bq_{fc}")
                       for fc in range(12)]

            # ---- long-lived attention-layout tensors ----
            qT = [qp.tile([128, N], F32R, name=f"qT{c}", tag=f"qT{c}")
                  for c in range(6)]
            kT = [qp.tile([128, N], F32R, name=f"kT{c}", tag=f"kT{c}")
                  for c in range(6)]
            vS = [qp.tile([128, 65 * H], F32R, name=f"vS{i}", tag=f"vS{i}")
                  for i in range(NT)]
            attnT = [qp.tile([128, N], F32R, name=f"attnT{p}", tag=f"attnT{p}")
                     for p in range(6)]

            from contextlib import ExitStack
            with ExitStack() as _xs:
                xp = tc.alloc_tile_pool(name="xw", bufs=1)
                xtp = tc.alloc_tile_pool(name="xtp", bufs=1)
                _xs.callback(lambda: xp.release())
                # ---- phase 0: load x (SWDGE-first = bandwidth priority),
                # transpose 8 per 2-bank psum tile (one group per j) ----
                xT = [xp.tile([128, N], F32R, name=f"xT{j}", tag=f"xT{j}")
                      for j in range(NE)]
                xts = {}
                for i in range(NT):
                    xt_i = xtp.tile([128, E], F32, name=f"xt{i}", tag=f"xt{i}")
                    nc.sync.dma_start(
                        out=xt_i, in_=x_d[i * 128:(i + 1) * 128, :])
                    xts[i] = xt_i
                # Q/K bias columns, queued on HWDGE after the x tiles
                for fc in range(12):
                    nc.sync.dma_start(
                        out=bq_cols[fc],
                        in_=bqkv_d[fc * 128:(fc + 1) * 128].rearrange(
                            "(p o) -> p o", o=1))
                # weights: V-bias row first, then V columns, then QK columns
                # SWDGE queue order gives x transfer priority over weights
                bv_row = xp.tile([1, E], F32R)
                nc.gpsimd.dma_start(
                    out=bv_row,
                    in_=bqkv_d[2 * E:3 * E].rearrange("(o f) -> o f", o=1))
                wqv = [xp.tile([128, E], F32R, name=f"wqv{j}", tag=f"wqv{j}")
                       for j in range(NE)]
                for j in range(NE):
                    nc.gpsimd.dma_start(
                        out=wqv[j], in_=wqkv_d[j * 128:(j + 1) * 128, 2 * E:])
                wqk = [xp.tile([128, 2 * E], F32R, name=f"wqk{j}", tag=f"wqk{j}")
                       for j in range(NE)]
                for j in range(NE):
                    nc.gpsimd.dma_start(
                        out=wqk[j], in_=wqkv_d[j * 128:(j + 1) * 128, 0:2 * E])
                for ig in range(2):
                    for j in range(NE):
                        pt = ps.tile([128, 512], F32, name=f"pt{ig}_{j}",
                                     tag=("s2", "mm", "u")[(ig * NE + j) % 3],
                                     bufs=2)
                        for ii in range(4):
                            i = ig * 4 + ii
                            nc.tensor.transpose(
                                pt[:, ii * 128:(ii + 1) * 128],
                                xts[i][:, j * 128:(j + 1) * 128], identf)
                        nc.vector.tensor_copy(
                            xT[j][:, ig * 512:(ig + 1) * 512], pt)

                xtp.release()
                # ---- phase 1a: V token-major with ones cols + bias ----
                onesH = xp.tile([128, H], F32)
                nc.vector.memset(onesH, 1.0)
                bv_bc = xp.tile([128, E], F32)
                for nf, (f0, fw) in enumerate(NFS):
                    pbv = ps.tile([128, 512], F32, name=f"pbv{nf}", tag="mm",
                                  bufs=2)
                    nc.tensor.matmul(pbv[:, :fw], ones1, bv_row[:, f0:f0 + fw],
                                     start=True, stop=True)
                    nc.vector.tensor_copy(bv_bc[:, f0:f0 + fw], pbv[:, :fw])
                for i in range(NT):
                    nc.vector.tensor_copy(
                        vS[i].rearrange("p (h c) -> p h c", c=65)[:, :, 64:65],
                        onesH.rearrange("p (h o) -> p h o", o=1))
                    for nf, (f0, fw) in enumerate(NFS):
                        pv = ps.tile([128, 512], F32, name=f"pv{i}_{nf}",
                                     tag=("s2", "mm", "u")[(i * 2 + nf) % 3],
                                     bufs=2)
                        for j in range(NE):
                            nc.tensor.matmul(
                                pv[:, :fw],
                                xT[j][:, i * 128:(i + 1) * 128],
                                wqv[j][:, f0:f0 + fw],
                                start=(j == 0), stop=(j == NE - 1))
                        nh, h0 = fw // D, f0 // D
                        nc.vector.tensor_add(
                            vS[i].rearrange("p (h c) -> p h c", c=65)
                                [:, h0:h0 + nh, 0:64],
                            pv[:, :fw].rearrange("p (h d) -> p h d", d=D),
                            bv_bc[:, f0:f0 + fw].rearrange(
                                "p (h d) -> p h d", d=D))

                # ---- phase 1b: Q^T / K^T feature-major pairs + bias ----
                for c in range(12):  # 0..5 -> qT, 6..11 -> kT
                    dst = qT[c] if c < 6 else kT[c - 6]
                    wcol0 = c * 128
                    for q in range(NQ):
                        pq = ps.tile([128, 512], F32, name=f"pq{c}_{q}",
                                     tag="mm", bufs=2)
                        for j in range(NE):
                            nc.tensor.matmul(
                                pq,
                                wqk[j][:, wcol0:wcol0 + 128],
                                xT[j][:, q * 512:(q + 1) * 512],
                                start=(j == 0), stop=(j == NE - 1))
                        nc.vector.tensor_scalar_add(
                            dst[:, q * 512:(q + 1) * 512], pq, bq_cols[c])

            # ---- phases 2+3: proj pool first so W_proj loads overlap
            # attention; exp pool released before proj matmuls need space ----
            with tc.tile_pool(name="proj", bufs=1) as pp:
                wp_sb = [pp.tile([128, E], F32R, name=f"wp{c}", tag=f"wp{c}")
                         for c in range(6)]
                for c in range(6):
                    nc.gpsimd.dma_start(
                        out=wp_sb[c], in_=wproj_d[c * 128:(c + 1) * 128, :])
                bp_row = pp.tile([1, E], F32R)
                nc.gpsimd.dma_start(
                    out=bp_row, in_=bproj_d[:].rearrange("(o f) -> o f", o=1))
                bp_bc = pp.tile([128, E], F32)
                for nf, (f0, fw) in enumerate(NFS):
                    pbp = ps.tile([128, 512], F32, name=f"pbp{nf}", tag="mm",
                                  bufs=2)
                    nc.tensor.matmul(pbp[:, :fw], ones1, bp_row[:, f0:f0 + fw],
                                     start=True, stop=True)
                    nc.vector.tensor_copy(bp_bc[:, f0:f0 + fw], pbp[:, :fw])
                _run_attention_and_proj(
                    nc, tc, ps, mybir, qT, kT, vS, attnT, ones65,
                    wp_sb, bp_bc, out_d)
    nc.compile()
    return nc


def _run_attention_and_proj(nc, tc2, ps, mybir, qT, kT, vS, attnT, ones65,
                            wp_sb, bp_bc, out_d):
    F32 = mybir.dt.float32
    F32R = mybir.dt.float32r
    EXP = mybir.ActivationFunctionType.Exp
    if True:
            with tc2.tile_pool(name="exp", bufs=1) as ep:
                expS_of = {}

                def emit_S(h):
                    c, r0 = h // 2, (h % 2) * 64
                    expS = [
                        ep.tile([128, N], F32R, name=f"expS{h}_{kc}",
                                tag="expS", bufs=16)
                        for kc in range(NT)]
                    expS_of[h] = expS
                    for kc in range(NT):
                        pss = ps.tile([128, N], F32, name=f"ps{h}_{kc}",
                                      tag="s2", bufs=2)
                        for q in range(NQ):
                            nc.tensor.matmul(
                                pss[:, q * 512:(q + 1) * 512],
                                kT[c][r0:r0 + 64, kc * 128:(kc + 1) * 128],
                                qT[c][r0:r0 + 64, q * 512:(q + 1) * 512],
                                start=True, stop=True)
                        nc.scalar.activation(expS[kc], pss, EXP,
                                             scale=float(SCALE))

                def emit_U(h):
                    c = h // 2
                    expS = expS_of.pop(h)
                    for q in range(NQ):
                        pu = ps.tile([65, 512], F32, name=f"pu{h}_{q}",
                                     tag="u", bufs=2)
                        for kc in range(NT):
                            nc.tensor.matmul(
                                pu,
                                vS[kc][:, h * 65:h * 65 + 65],
                                expS[kc][:, q * 512:(q + 1) * 512],
                                start=(kc == 0), stop=(kc == NT - 1))
                        rz = ep.tile([65, 512], F32R, name=f"rz{h}_{q}",
                                     tag="rz", bufs=2)
                        with nc.allow_low_precision(reason="invZ f32r bcast"):
                            nc.vector.reciprocal(rz[64:65, :], pu[64:65, :])
                        pb = ps.tile([128, 512], F32, name=f"pb{h}_{q}",
                                     tag="mm", bufs=2)
                        nc.tensor.matmul(
                            pb[0:64, :], ones65[64:65, :], rz[64:65, :],
                            start=True, stop=True)
                        pbs = ep.tile([64, 512], F32, name=f"pbs{h}_{q}",
                                      tag="pbs", bufs=2)
                        nc.vector.tensor_copy(pbs, pb[0:64, :])
                        if h % 2 == 0:
                            nc.vector.tensor_mul(
                                attnT[c][0:64, q * 512:(q + 1) * 512],
                                pu[0:64, :], pbs)
                        else:
                            tmp = ep.tile([64, 512], F32R, name=f"tmp{h}_{q}",
                                          tag="tmp", bufs=2)
                            nc.vector.tensor_mul(tmp, pu[0:64, :], pbs)
                            nc.sync.dma_start(
                                out=attnT[c][64:128, q * 512:(q + 1) * 512],
                                in_=tmp)


                for h in range(H):
                    emit_S(h)
                    if h > 0:
                        emit_U(h - 1)
                emit_U(H - 1)

            # ---- phase 3: output projection ----
            with tc2.tile_pool(name="osb", bufs=1) as op:
                for i in range(NT):
                    o_sb = op.tile([128, E], F32, name=f"o{i}", tag="o", bufs=4)
                    for nf, (f0, fw) in enumerate(NFS):
                        po = ps.tile([128, 512], F32, name=f"po{i}_{nf}",
                                     tag=("s2", "mm", "u")[(i * 2 + nf) % 3],
                                     bufs=2)
                        for c in range(6):
                            nc.tensor.matmul(
                                po[:, :fw],
                                attnT[c][:, i * 128:(i + 1) * 128],
                                wp_sb[c][:, f0:f0 + fw],
                                start=(c == 0), stop=(c == 5))
                        nc.vector.tensor_add(
                            o_sb[:, f0:f0 + fw], po[:, :fw],
                            bp_bc[:, f0:f0 + fw])
                    nc.sync.dma_start(
                        out=out_d[i * 128:(i + 1) * 128, :], in_=o_sb)


_NC_CACHE = None


def kernel(x, W_qkv, b_qkv, W_proj, b_proj):
    from concourse.bass_utils import run_bass_kernel_spmd

    global _NC_CACHE
    if _NC_CACHE is None:
        _NC_CACHE = _build()
    nc = _NC_CACHE

    x = np.ascontiguousarray(np.asarray(x, dtype=np.float32))
    W_qkv = np.ascontiguousarray(np.asarray(W_qkv, dtype=np.float32))
    b_qkv = np.ascontiguousarray(np.asarray(b_qkv, dtype=np.float32))
    W_proj = np.ascontiguousarray(np.asarray(W_proj, dtype=np.float32))
    b_proj = np.ascontiguousarray(np.asarray(b_proj, dtype=np.float32))

    in_maps = [
        {"x": x[b], "W_qkv": W_qkv, "b_qkv": b_qkv,
         "W_proj": W_proj, "b_proj": b_proj}
        for b in range(B)
    ]
    res = run_bass_kernel_spmd(nc, in_maps, core_ids=list(range(B)))
    return np.stack([np.asarray(res.results[b]["out"]) for b in range(B)])



# revision 5
# speedup vs baseline: 1.1025x; 1.1025x over previous
"""Multi-head attention block on 8 Trainium2 NeuronCores.

Problem: B=8, N=1024, E=768, H=12, D=64 attention (QKV proj -> softmax(QK^T/8)V
-> output proj), fp32 I/O. Data parallel over batch: core b owns batch b.

v2 design (all-bf16 matmuls; Act-saturating schedule):
  - x and all weights stream in via SWDGE casting DMAs (fp32 DRAM -> bf16
    SBUF), no staging or engine cast passes.
  - xT via PE transpose of bf16 x chunks (1 cyc/row), evac on Pool.
  - S^T[k,q] per head: two 512-wide bf16 matmuls into a [128,1024] psum
    (contraction d=64 at partition base (h%2)*64); exp on Act -> bf16 expS.
  - U restructured: stationary = expS chunk [128k,128q], moving = V [128k,65]
    (64 dims + ones column) -> psum U^T[q, 65] accumulated over k chunks.
    Column 64 is the softmax denominator Z; invZ = reciprocal([128,1]) is a
    per-partition scalar, so attn = U^T * invZ is one DVE tensor_scalar op.
    This halves U's PE rows vs the v1 layout and kills the PE invZ broadcast.
  - attn rows (token-major) -> attnT (feature-major) via HWDGE XBAR DMA
    transposes ([128,128] bf16 blocks, 112ns each, zero PE).
  - Output proj split: heads 0..5 (attnT blocks 0..2) projected during late
    attention as PE filler; heads 6..11 in the tail, accumulated into the
    same SBUF f32 tile; bias added at the first evac.
  - Emission interleaves S psum fills with QK/V/U/proj filler units so the
    Act engine (exp is 99.6us of work) never starves while PE stays busy.
"""
import numpy as np

B, N, E, H, D = 8, 1024, 768, 12, 64
SCALE = D ** -0.5
NT = N // 128   # token chunks (8)
NE = E // 128   # embed chunks (6)
NQ = N // 512   # moving-dim tiles (2)
NFS = [(0, 512), (512, 256)]  # free-dim split of E for matmuls
PROJ_SPLIT = 3  # attnT blocks 0..2 in projA (during attention), 3..5 in tail


def _build():
    import concourse.bacc as bacc
    import concourse.mybir as mybir
    import concourse.tile as tile
    from concourse.masks import make_identity

    F32 = mybir.dt.float32
    BF16 = mybir.dt.bfloat16
    EXP = mybir.ActivationFunctionType.Exp

    nc = bacc.Bacc("TRN2", target_bir_lowering=False)
    x_d = nc.declare_dram_parameter("x", [N, E], F32, isOutput=False)
    wqkv_d = nc.declare_dram_parameter("W_qkv", [E, 3 * E], F32, isOutput=False)
    bqkv_d = nc.declare_dram_parameter("b_qkv", [3 * E], F32, isOutput=False)
    wproj_d = nc.declare_dram_parameter("W_proj", [E, E], F32, isOutput=False)
    bproj_d = nc.declare_dram_parameter("b_proj", [E], F32, isOutput=False)
    out_d = nc.declare_dram_parameter("out", [N, E], F32, isOutput=True)

    with tile.TileContext(nc) as tc:
        with (
            tc.tile_pool(name="const", bufs=1) as cp,
            tc.tile_pool(name="main", bufs=1) as qp,
            tc.tile_pool(name="psum", bufs=1, space="PSUM") as ps,
        ):
            # ---- constants ----
            identf = cp.tile([128, 128], F32)
            make_identity(nc, identf)
            identb = cp.tile([128, 128], BF16)
            nc.vector.tensor_copy(identb, identf)
            ones1 = cp.tile([1, 128], BF16)
            nc.vector.memset(ones1, 1.0)
            bq_cols = [cp.tile([128, 1], F32, name=f"bq_{fc}", tag=f"bq_{fc}")
                       for fc in range(12)]

            # ---- long-lived tensors ----
            qT = [qp.tile([128, N], BF16, name=f"qT{c}", tag=f"qT{c}")
                  for c in range(6)]
            kT = [qp.tile([128, N], BF16, name=f"kT{c}", tag=f"kT{c}")
                  for c in range(6)]
            vS = [qp.tile([128, 65 * H], BF16, name=f"vS{i}", tag=f"vS{i}")
                  for i in range(NT)]
            attnS = [qp.tile([128, E], BF16, name=f"atS{i}", tag=f"atS{i}")
                     for i in range(NT)]
            attnT = qp.tile([128, NE * N], BF16)  # [128, (c, 1024)]
            attnTv = attnT.rearrange("p (c n) -> p c n", n=N)
            wp = [qp.tile([128, E], BF16, name=f"wp{c}", tag=f"wp{c}")
                  for c in range(6)]
            bv_bc = qp.tile([128, E], F32)
            bp_bc = qp.tile([128, E], F32)
            bv_row = qp.tile([1, E], BF16)
            bp_row = qp.tile([1, E], BF16)
            o_acc = [qp.tile([128, E], F32, name=f"oa{i}", tag=f"oa{i}")
                     for i in range(NT)]

            # expS pool: tiles [128, N] bf16; 3 heads alive (lag 2)
            ep = tc.alloc_tile_pool(name="exp", bufs=1)
            iz = tc.alloc_tile_pool(name="iz", bufs=1)

            # scoped pools: xp2 (xT/W, released at h=5), xp1 (xb, released
            # right after the transposes); LIFO order xp1 before xp2.
            xp2 = tc.alloc_tile_pool(name="xw2", bufs=1)
            xT = xp2.tile([128, NE * N], BF16)  # [128, (j, 1024 tok)]
            xTv = xT.rearrange("p (j n) -> p j n", n=N)
            wqk = [xp2.tile([128, 2 * E], BF16, name=f"wqk{j}", tag=f"wqk{j}")
                   for j in range(NE)]
            wv = [xp2.tile([128, E], BF16, name=f"wv{j}", tag=f"wv{j}")
                  for j in range(NE)]
            xp1 = tc.alloc_tile_pool(name="xw1", bufs=1)
            xb = [xp1.tile([128, E], BF16, name=f"xb{i}", tag=f"xb{i}")
                  for i in range(NT)]

            # ---- DMAs: bias rows (tiny) then x (critical), then weights ----
            nc.gpsimd.dma_start(
                out=bv_row,
                in_=bqkv_d[2 * E:3 * E].rearrange("(o f) -> o f", o=1))
            nc.gpsimd.dma_start(
                out=bp_row, in_=bproj_d[:].rearrange("(o f) -> o f", o=1))
            for i in range(NT):
                nc.gpsimd.dma_start(out=xb[i], in_=x_d[i * 128:(i + 1) * 128, :])
            for fc in range(12):
                nc.sync.dma_start(
                    out=bq_cols[fc],
                    in_=bqkv_d[fc * 128:(fc + 1) * 128].rearrange(
                        "(p o) -> p o", o=1))
            for j in range(NE):
                nc.gpsimd.dma_start(
                    out=wqk[j], in_=wqkv_d[j * 128:(j + 1) * 128, 0:2 * E])
            for j in range(NE):
                nc.gpsimd.dma_start(
                    out=wv[j], in_=wqkv_d[j * 128:(j + 1) * 128, 2 * E:])
            for c in range(6):
                nc.gpsimd.dma_start(
                    out=wp[c], in_=wproj_d[c * 128:(c + 1) * 128, :])

            # ---- xT: PE transpose of bf16 chunks ----
            for i in range(NT):
                pt = ps.tile([128, 1024], BF16, name=f"pt{i}", tag="s2",
                             bufs=2)
                for j in range(NE):
                    nc.tensor.transpose(
                        pt[:, j * 128:(j + 1) * 128],
                        xb[i][:, j * 128:(j + 1) * 128], identb)
                nc.vector.tensor_copy(
                    xTv[:, :, i * 128:(i + 1) * 128],
                    pt[:, :NE * 128].rearrange("p (j t) -> p j t", t=128))
            xp1.release()

            # ---- bias broadcast rows -> [128, E] via ones outer product ----
            for nf, (f0, fw) in enumerate(NFS):
                pbv = ps.tile([128, 512], F32, name=f"pbv{nf}", tag="mm",
                              bufs=2)
                nc.tensor.matmul(pbv[:, :fw], ones1, bv_row[:, f0:f0 + fw],
                                 start=True, stop=True)
                nc.vector.tensor_copy(bv_bc[:, f0:f0 + fw], pbv[:, :fw])
                pbp = ps.tile([128, 512], F32, name=f"pbp{nf}", tag="mm",
                              bufs=2)
                nc.tensor.matmul(pbp[:, :fw], ones1, bp_row[:, f0:f0 + fw],
                                 start=True, stop=True)
                nc.vector.tensor_copy(bp_bc[:, f0:f0 + fw], pbp[:, :fw])

            # ---- vS ones columns ----
            for i in range(NT):
                nc.vector.memset(
                    vS[i].rearrange("p (h c) -> p h c", c=65)[:, :, 64:65],
                    1.0)

            # ================= emission units =================
            def emit_qk_unit(t, q):
                """One (feature-tile, 512-token-half) of Q or K projection."""
                dst = qT[t] if t < 6 else kT[t - 6]
                wcol0 = t * 128
                pq = ps.tile([128, 512], F32, name=f"pq{t}_{q}", tag="mm",
                             bufs=2)
                for j in range(NE):
                    nc.tensor.matmul(
                        pq,
                        wqk[j][:, wcol0:wcol0 + 128],
                        xTv[:, j, q * 512:(q + 1) * 512],
                        start=(j == 0), stop=(j == NE - 1))
                nc.vector.tensor_scalar_add(
                    dst[:, q * 512:(q + 1) * 512], pq, bq_cols[t])

            def emit_v_unit(i, nf):
                """One (token-chunk, free-half) of the V projection."""
                f0, fw = NFS[nf]
                pv = ps.tile([128, 512], F32, name=f"pv{i}_{nf}", tag="mm",
                             bufs=2)
                for j in range(NE):
                    nc.tensor.matmul(
                        pv[:, :fw],
                        xTv[:, j, i * 128:(i + 1) * 128],
                        wv[j][:, f0:f0 + fw],
                        start=(j == 0), stop=(j == NE - 1))
                nh, h0 = fw // D, f0 // D
                nc.vector.tensor_add(
                    vS[i].rearrange("p (h c) -> p h c", c=65)
                        [:, h0:h0 + nh, 0:64],
                    pv[:, :fw].rearrange("p (h d) -> p h d", d=D),
                    bv_bc[:, f0:f0 + fw].rearrange("p (h d) -> p h d", d=D))

            expS_of = {}

            def emit_s_unit(h, kc):
                """S^T[k-chunk, all q] for one head + exp -> bf16 expS."""
                c, r0 = h // 2, (h % 2) * 64
                if kc == 0:
                    expS_of[h] = [
                        ep.tile([128, N], BF16, name=f"eS{h}_{k2}",
                                tag="expS", bufs=24)
                        for k2 in range(NT)]
                pss = ps.tile([128, N], F32, name=f"ps{h}_{kc}", tag="s2",
                              bufs=2)
                for q in range(NQ):
                    nc.tensor.matmul(
                        pss[:, q * 512:(q + 1) * 512],
                        kT[c][r0:r0 + 64, kc * 128:(kc + 1) * 128],
                        qT[c][r0:r0 + 64, q * 512:(q + 1) * 512],
                        start=True, stop=True)
                nc.scalar.activation(expS_of[h][kc], pss, EXP,
                                     scale=float(SCALE))

            def emit_u_unit(h, qc):
                """U^T[q-chunk, 65] for one head; normalize into attnS."""
                expS = expS_of[h]
                pu = ps.tile([128, 512], F32, name=f"pu{h}_{qc}", tag="u",
                             bufs=2)
                for kc in range(NT):
                    nc.tensor.matmul(
                        pu[:, 0:65],
                        expS[kc][:, qc * 128:(qc + 1) * 128],
                        vS[kc][:, h * 65:h * 65 + 65],
                        start=(kc == 0), stop=(kc == NT - 1))
                invz = iz.tile([128, 1], F32, name=f"iv{h}_{qc}", tag="iz",
                               bufs=3)
                nc.vector.reciprocal(invz, pu[:, 64:65])
                nc.vector.tensor_scalar_mul(
                    attnS[qc][:, h * D:(h + 1) * D], pu[:, 0:64], invz)

            def emit_transp(qc, half):
                """DMA-transpose 3 [128,128] bf16 blocks into attnT."""
                c0 = half * PROJ_SPLIT
                nc.sync.dma_start_transpose(
                    attnTv[:, c0:c0 + PROJ_SPLIT, qc * 128:(qc + 1) * 128],
                    attnS[qc][:, c0 * 128:(c0 + PROJ_SPLIT) * 128])

            def emit_proj_unit(i, nf, phase):
                """Half-row of output proj; phase 0 = blocks 0..2 (+bias),
                phase 1 = blocks 3..5 (+accumulate into o_acc) then store."""
                f0, fw = NFS[nf]
                cs = range(PROJ_SPLIT) if phase == 0 else range(PROJ_SPLIT, 6)
                po = ps.tile([128, 512], F32, name=f"po{i}_{nf}_{phase}",
                             tag="mm", bufs=2)
                for ci, c in enumerate(cs):
                    nc.tensor.matmul(
                        po[:, :fw],
                        attnTv[:, c, i * 128:(i + 1) * 128],
                        wp[c][:, f0:f0 + fw],
                        start=(ci == 0), stop=(ci == len(cs) - 1))
                if phase == 0:
                    nc.vector.tensor_add(
                        o_acc[i][:, f0:f0 + fw], po[:, :fw],
                        bp_bc[:, f0:f0 + fw])
                else:
                    nc.vector.tensor_add(
                        o_acc[i][:, f0:f0 + fw], po[:, :fw],
                        o_acc[i][:, f0:f0 + fw])

            # ================= schedule =================
            # QK feature-tile pairs: chunk c covers qT[c] (t=c) and kT[c]
            # (t=6+c). Chunk 0 up front; chunk c+1 emitted during head pair c.
            for t in (0, 6):
                for q in range(NQ):
                    emit_qk_unit(t, q)

            for h in range(H):
                c = h // 2
                fillers = []
                if h < 5:  # QK chunk h+1
                    for t in (h + 1, 6 + h + 1):
                        for q in range(NQ):
                            fillers.append(("qk", t, q))
                if h in (1, 2):  # V projection (wv lands ~22us)
                    for i in range(4 * (h - 1), 4 * (h - 1) + 4):
                        fillers.append(("v", i, 0))
                        fillers.append(("v", i, 1))
                if h >= 2:  # U for head h-2 (after V units at h=2)
                    for qc in range(NT):
                        fillers.append(("u", h - 2, qc))
                if h == 5:
                    fillers.append(("xfree",))
                if h >= 8:  # projA as late filler (needs attnT blocks 0..2)
                    for i in range(2 * (h - 8), 2 * (h - 8) + 2):
                        fillers.append(("pa", i, 0))
                        fillers.append(("pa", i, 1))

                def drain(k):
                    for _ in range(k):
                        if not fillers:
                            return
                        f = fillers.pop(0)
                        if f[0] == "v":
                            emit_v_unit(f[1], f[2])
                        elif f[0] == "qk":
                            emit_qk_unit(f[1], f[2])
                        elif f[0] == "u":
                            emit_u_unit(f[1], f[2])
                            if f[1] == 5:
                                # heads 0..5 done for this q-chunk:
                                # transpose attnT blocks 0..2
                                emit_transp(f[2], 0)
                        elif f[0] == "pa":
                            emit_proj_unit(f[1], f[2], 0)
                        elif f[0] == "xfree":
                            xp2.release()

                per = (len(fillers) + NT - 1) // NT
                for kc in range(NT):
                    emit_s_unit(h, kc)
                    drain(per)
                drain(len(fillers))
                if h >= 2:
                    expS_of.pop(h - 2)

            # ---- tail: U(10), U(11) + attnT blocks 3..5 + projB + store ----
            for qc in range(NT):
                emit_u_unit(10, qc)
            for qc in range(NT):
                emit_u_unit(11, qc)
                emit_transp(qc, 1)
            for qc in range(NT):
                emit_proj_unit(qc, 0, 1)
                emit_proj_unit(qc, 1, 1)
                nc.sync.dma_start(
                    out=out_d[qc * 128:(qc + 1) * 128, :], in_=o_acc[qc])
            iz.release()
            ep.release()
    nc.compile()
    return nc


_NC_CACHE = None


def kernel(x, W_qkv, b_qkv, W_proj, b_proj):
    from concourse.bass_utils import run_bass_kernel_spmd

    global _NC_CACHE
    if _NC_CACHE is None:
        _NC_CACHE = _build()
    nc = _NC_CACHE

    x = np.ascontiguousarray(np.asarray(x, dtype=np.float32))
    W_qkv = np.ascontiguousarray(np.asarray(W_qkv, dtype=np.float32))
    b_qkv = np.ascontiguousarray(np.asarray(b_qkv, dtype=np.float32))
    W_proj = np.ascontiguousarray(np.asarray(W_proj, dtype=np.float32))
    b_proj = np.ascontiguousarray(np.asarray(b_proj, dtype=np.float32))

    in_maps = [
        {"x": x[b], "W_qkv": W_qkv, "b_qkv": b_qkv,
         "W_proj": W_proj, "b_proj": b_proj}
        for b in range(B)
    ]
    res = run_bass_kernel_spmd(nc, in_maps, core_ids=list(range(B)))
    return np.stack([np.asarray(res.results[b]["out"]) for b in range(B)])


# revision 7
# speedup vs baseline: 1.1362x; 1.0305x over previous
"""Multi-head attention block on 8 Trainium2 NeuronCores.

Problem: B=8, N=1024, E=768, H=12, D=64 attention (QKV proj -> softmax(QK^T/8)V
-> output proj), fp32 I/O. Data parallel over batch: core b owns batch b.

v2 design (all-bf16 matmuls; Act-saturating schedule):
  - x and all weights stream in via SWDGE casting DMAs (fp32 DRAM -> bf16
    SBUF), no staging or engine cast passes.
  - xT via PE transpose of bf16 x chunks (1 cyc/row), evac on Pool.
  - S^T[k,q] per head: two 512-wide bf16 matmuls into a [128,1024] psum
    (contraction d=64 at partition base (h%2)*64); exp on Act -> bf16 expS.
  - U restructured: stationary = expS chunk [128k,128q], moving = V [128k,65]
    (64 dims + ones column) -> psum U^T[q, 65] accumulated over k chunks.
    Column 64 is the softmax denominator Z; invZ = reciprocal([128,1]) is a
    per-partition scalar, so attn = U^T * invZ is one DVE tensor_scalar op.
    This halves U's PE rows vs the v1 layout and kills the PE invZ broadcast.
  - attn rows (token-major) -> attnT (feature-major) via HWDGE XBAR DMA
    transposes ([128,128] bf16 blocks, 112ns each, zero PE).
  - Output proj split: heads 0..5 (attnT blocks 0..2) projected during late
    attention as PE filler; heads 6..11 in the tail, accumulated into the
    same SBUF f32 tile; bias added at the first evac.
  - Emission interleaves S psum fills with QK/V/U/proj filler units so the
    Act engine (exp is 99.6us of work) never starves while PE stays busy.
"""
import numpy as np

B, N, E, H, D = 8, 1024, 768, 12, 64
SCALE = D ** -0.5
NT = N // 128   # token chunks (8)
NE = E // 128   # embed chunks (6)
NQ = N // 512   # moving-dim tiles (2)
NFS = [(0, 512), (512, 256)]  # free-dim split of E for matmuls
PROJ_SPLIT = 3  # attnT blocks 0..2 in projA (during attention), 3..5 in tail


def _build():
    import concourse.bacc as bacc
    import concourse.mybir as mybir
    import concourse.tile as tile
    from concourse.masks import make_identity

    F32 = mybir.dt.float32
    BF16 = mybir.dt.bfloat16
    F8 = mybir.dt.float8e4
    EXP = mybir.ActivationFunctionType.Exp
    DR = mybir.MatmulPerfMode.DoubleRow
    MUL = mybir.AluOpType.mult
    ADD = mybir.AluOpType.add

    nc = bacc.Bacc("TRN2", target_bir_lowering=False)
    x_d = nc.declare_dram_parameter("x", [N, E], F32, isOutput=False)
    wqkvh_d = nc.declare_dram_parameter("W_qkvh", [E, 3 * E], F8, isOutput=False)
    wqkvl_d = nc.declare_dram_parameter("W_qkvl", [E, 3 * E], F8, isOutput=False)
    bqkv_d = nc.declare_dram_parameter("b_qkv", [3 * E], F32, isOutput=False)
    wproj_d = nc.declare_dram_parameter("W_proj", [E, E], F32, isOutput=False)
    bproj_d = nc.declare_dram_parameter("b_proj", [E], F32, isOutput=False)
    out_d = nc.declare_dram_parameter("out", [N, E], F32, isOutput=True)

    with tile.TileContext(nc) as tc:
        with (
            tc.tile_pool(name="const", bufs=1) as cp,
            tc.tile_pool(name="main", bufs=1) as qp,
            tc.tile_pool(name="psum", bufs=1, space="PSUM") as ps,
        ):
            # ---- constants ----
            identf = cp.tile([128, 128], F32)
            make_identity(nc, identf)
            ident32 = cp.tile([128, 128], BF16)
            nc.vector.tensor_scalar_mul(ident32, identf, 32.0)
            ones1 = cp.tile([1, 128], BF16)
            nc.vector.memset(ones1, 1.0)
            ones32k = cp.tile([1, 128], BF16)
            nc.vector.memset(ones32k, 32768.0)
            bq_cols = [cp.tile([128, 1], F32, name=f"bq_{fc}", tag=f"bq_{fc}")
                       for fc in range(12)]

            # ---- long-lived tensors ----
            qT = [qp.tile([128, N], BF16, name=f"qT{c}", tag=f"qT{c}")
                  for c in range(6)]
            kT = [qp.tile([128, N], BF16, name=f"kT{c}", tag=f"kT{c}")
                  for c in range(6)]
            vS = [qp.tile([128, 65 * H], BF16, name=f"vS{i}", tag=f"vS{i}")
                  for i in range(NT)]
            attnS = [qp.tile([128, E], BF16, name=f"atS{i}", tag=f"atS{i}")
                     for i in range(NT)]
            attnT = qp.tile([128, NE * N], BF16)  # [128, (c, 1024)]
            attnTv = attnT.rearrange("p (c n) -> p c n", n=N)
            wp = [qp.tile([128, E], BF16, name=f"wp{c}", tag=f"wp{c}")
                  for c in range(6)]
            bv_bc = qp.tile([128, E], F32)
            bp_bc = qp.tile([128, E], F32)
            bv_row = qp.tile([1, E], BF16)
            bp_row = qp.tile([1, E], BF16)
            o_acc = [qp.tile([128, E], F32, name=f"oa{i}", tag=f"oa{i}")
                     for i in range(NT)]

            # expS pool: tiles [128, N] bf16; 3 heads alive (lag 2)
            ep = tc.alloc_tile_pool(name="exp", bufs=1)
            iz = tc.alloc_tile_pool(name="iz", bufs=1)

            # scoped pools: xp2 (xT/W, released at h=5), xp1 (xb, released
            # right after the transposes); LIFO order xp1 before xp2.
            xp2 = tc.alloc_tile_pool(name="xw2", bufs=1)
            xH = xp2.tile([128, NE * N], F8)  # [128, (j, 1024 tok)] = 32*x hi
            xL = xp2.tile([128, NE * N], F8)  # residual
            xHv = xH.rearrange("p (j n) -> p j n", n=N)
            xLv = xL.rearrange("p (j n) -> p j n", n=N)
            # weight pair tiles: pair p = W rows 256p..256p+256 as [128, 2, f]
            wqkh = [xp2.tile([128, 2 * 1536], F8, name=f"wqh{p}", tag=f"wqh{p}")
                    for p in range(3)]
            wqkl = [xp2.tile([128, 2 * 1536], F8, name=f"wql{p}", tag=f"wql{p}")
                    for p in range(3)]
            wvh = [xp2.tile([128, 2 * E], F8, name=f"wvh{p}", tag=f"wvh{p}")
                   for p in range(3)]
            wvl = [xp2.tile([128, 2 * E], F8, name=f"wvl{p}", tag=f"wvl{p}")
                   for p in range(3)]
            xp1 = tc.alloc_tile_pool(name="xw1", bufs=1)
            xb = [xp1.tile([128, E], BF16, name=f"xb{i}", tag=f"xb{i}")
                  for i in range(NT)]

            # ---- DMAs: bias rows (tiny) then x (critical), then weights ----
            nc.gpsimd.dma_start(
                out=bv_row,
                in_=bqkv_d[2 * E:3 * E].rearrange("(o f) -> o f", o=1))
            nc.gpsimd.dma_start(
                out=bp_row, in_=bproj_d[:].rearrange("(o f) -> o f", o=1))
            for i in range(NT):
                nc.gpsimd.dma_start(out=xb[i], in_=x_d[i * 128:(i + 1) * 128, :])
            for fc in range(12):
                nc.sync.dma_start(
                    out=bq_cols[fc],
                    in_=bqkv_d[fc * 128:(fc + 1) * 128].rearrange(
                        "(p o) -> p o", o=1))
            for p in range(3):
                for w_t, w_d, c0, cw in (
                    (wqkh[p], wqkvh_d, 0, 1536), (wqkl[p], wqkvl_d, 0, 1536),
                    (wvh[p], wqkvh_d, 1536, E), (wvl[p], wqkvl_d, 1536, E),
                ):
                    nc.sync.dma_start(
                        out=w_t.rearrange("p (t f) -> p t f", t=2),
                        in_=w_d[256 * p:256 * (p + 1), c0:c0 + cw].rearrange(
                            "(t k) f -> k t f", t=2))
            for c in range(6):
                nc.gpsimd.dma_start(
                    out=wp[c], in_=wproj_d[c * 128:(c + 1) * 128, :])

            # ---- xT: transpose-with-scale (32*x) then fp8 hi/lo split ----
            for i in range(NT):
                pt = ps.tile([128, 1024], F32, name=f"pt{i}", tag="s2",
                             bufs=2)
                for j in range(NE):
                    nc.tensor.matmul(
                        pt[:, j * 128:(j + 1) * 128],
                        xb[i][:, j * 128:(j + 1) * 128], ident32,
                        start=True, stop=True)
                hi = xHv[:, :, i * 128:(i + 1) * 128]
                ptv = pt[:, :NE * 128].rearrange("p (j t) -> p j t", t=128)
                nc.vector.tensor_copy(hi, ptv)
                nc.vector.tensor_sub(
                    xLv[:, :, i * 128:(i + 1) * 128], ptv, hi)
            xp1.release()

            # ---- bias broadcast rows -> [128, E] via ones outer product ----
            for nf, (f0, fw) in enumerate(NFS):
                pbv = ps.tile([128, 512], F32, name=f"pbv{nf}", tag="mm",
                              bufs=2)
                nc.tensor.matmul(pbv[:, :fw], ones32k, bv_row[:, f0:f0 + fw],
                                 start=True, stop=True)
                nc.vector.tensor_copy(bv_bc[:, f0:f0 + fw], pbv[:, :fw])
                pbp = ps.tile([128, 512], F32, name=f"pbp{nf}", tag="mm",
                              bufs=2)
                nc.tensor.matmul(pbp[:, :fw], ones1, bp_row[:, f0:f0 + fw],
                                 start=True, stop=True)
                nc.vector.tensor_copy(bp_bc[:, f0:f0 + fw], pbp[:, :fw])

            # ---- vS ones columns ----
            for i in range(NT):
                nc.vector.memset(
                    vS[i].rearrange("p (h c) -> p h c", c=65)[:, :, 64:65],
                    32768.0)

            # ================= emission units =================
            def emit_qk_unit(t, q):
                """One (feature-tile, 512-token-half) of Q or K projection.
                Split-fp8 DoubleRow: (xh+xl)@(wh+wl), xl@wl dropped."""
                dst = qT[t] if t < 6 else kT[t - 6]
                wcol0 = t * 128
                pq = ps.tile([128, 512], F32, name=f"pq{t}_{q}", tag="mm",
                             bufs=2)
                terms = [(wqkh, xHv), (wqkh, xLv), (wqkl, xHv)]
                nmm = 3 * len(terms)
                mi = 0
                for p in range(3):
                    for w_t, x_v in terms:
                        wv3 = w_t[p].rearrange("p (t f) -> p t f", t=2)
                        nc.tensor.matmul(
                            pq,
                            wv3[:, :, wcol0:wcol0 + 128],
                            x_v[:, 2 * p:2 * p + 2, q * 512:(q + 1) * 512],
                            start=(mi == 0), stop=(mi == nmm - 1),
                            perf_mode=DR)
                        mi += 1
                nc.vector.tensor_scalar(
                    out=dst[:, q * 512:(q + 1) * 512], in0=pq,
                    scalar1=float(2.0 ** -15), scalar2=bq_cols[t],
                    op0=MUL, op1=ADD)

            def emit_v_unit(i, nf):
                """One (token-chunk, free-half) of the V projection."""
                f0, fw = NFS[nf]
                pv = ps.tile([128, 512], F32, name=f"pv{i}_{nf}", tag="mm",
                             bufs=2)
                terms = [(xHv, wvh), (xHv, wvl), (xLv, wvh)]
                nmm = 3 * len(terms)
                mi = 0
                for p in range(3):
                    for x_v, w_t in terms:
                        wv3 = w_t[p].rearrange("p (t f) -> p t f", t=2)
                        nc.tensor.matmul(
                            pv[:, :fw],
                            x_v[:, 2 * p:2 * p + 2, i * 128:(i + 1) * 128],
                            wv3[:, :, f0:f0 + fw],
                            start=(mi == 0), stop=(mi == nmm - 1),
                            perf_mode=DR)
                        mi += 1
                nh, h0 = fw // D, f0 // D
                nc.vector.tensor_add(
                    vS[i].rearrange("p (h c) -> p h c", c=65)
                        [:, h0:h0 + nh, 0:64],
                    pv[:, :fw].rearrange("p (h d) -> p h d", d=D),
                    bv_bc[:, f0:f0 + fw].rearrange("p (h d) -> p h d", d=D))

            expS_of = {}

            def emit_s_unit(h, kc):
                """S^T[k-chunk, all q] for one head + exp -> bf16 expS."""
                c, r0 = h // 2, (h % 2) * 64
                if kc == 0:
                    expS_of[h] = [
                        ep.tile([128, N], BF16, name=f"eS{h}_{k2}",
                                tag="expS", bufs=24)
                        for k2 in range(NT)]
                pss = ps.tile([128, N], F32, name=f"ps{h}_{kc}", tag="s2",
                              bufs=2)
                for q in range(NQ):
                    nc.tensor.matmul(
                        pss[:, q * 512:(q + 1) * 512],
                        kT[c][r0:r0 + 64, kc * 128:(kc + 1) * 128],
                        qT[c][r0:r0 + 64, q * 512:(q + 1) * 512],
                        start=True, stop=True)
                nc.scalar.activation(expS_of[h][kc], pss, EXP,
                                     scale=float(SCALE))

            def emit_u_unit(h, qc):
                """U^T[q-chunk, 65] for one head; normalize into attnS."""
                expS = expS_of[h]
                pu = ps.tile([128, 512], F32, name=f"pu{h}_{qc}", tag="u",
                             bufs=2)
                for kc in range(NT):
                    nc.tensor.matmul(
                        pu[:, 0:65],
                        expS[kc][:, qc * 128:(qc + 1) * 128],
                        vS[kc][:, h * 65:h * 65 + 65],
                        start=(kc == 0), stop=(kc == NT - 1))
                invz = iz.tile([128, 1], F32, name=f"iv{h}_{qc}", tag="iz",
                               bufs=3)
                nc.vector.reciprocal(invz, pu[:, 64:65])
                nc.vector.tensor_scalar_mul(
                    attnS[qc][:, h * D:(h + 1) * D], pu[:, 0:64], invz)

            def emit_transp(qc, half):
                """DMA-transpose 3 [128,128] bf16 blocks into attnT."""
                c0 = half * PROJ_SPLIT
                nc.sync.dma_start_transpose(
                    attnTv[:, c0:c0 + PROJ_SPLIT, qc * 128:(qc + 1) * 128],
                    attnS[qc][:, c0 * 128:(c0 + PROJ_SPLIT) * 128])

            def emit_proj_unit(i, nf, phase):
                """Half-row of output proj; phase 0 = blocks 0..2 (+bias),
                phase 1 = blocks 3..5 (+accumulate into o_acc) then store."""
                f0, fw = NFS[nf]
                cs = range(PROJ_SPLIT) if phase == 0 else range(PROJ_SPLIT, 6)
                po = ps.tile([128, 512], F32, name=f"po{i}_{nf}_{phase}",
                             tag="mm", bufs=2)
                for ci, c in enumerate(cs):
                    nc.tensor.matmul(
                        po[:, :fw],
                        attnTv[:, c, i * 128:(i + 1) * 128],
                        wp[c][:, f0:f0 + fw],
                        start=(ci == 0), stop=(ci == len(cs) - 1))
                if phase == 0:
                    nc.vector.tensor_add(
                        o_acc[i][:, f0:f0 + fw], po[:, :fw],
                        bp_bc[:, f0:f0 + fw])
                else:
                    nc.vector.tensor_add(
                        o_acc[i][:, f0:f0 + fw], po[:, :fw],
                        o_acc[i][:, f0:f0 + fw])

            # ================= schedule =================
            # QK feature-tile pairs: chunk c covers qT[c] (t=c) and kT[c]
            # (t=6+c). Chunk 0 up front; chunk c+1 emitted during head pair c.
            for t in (0, 6):
                for q in range(NQ):
                    emit_qk_unit(t, q)

            for h in range(H):
                c = h // 2
                fillers = []
                if h < 5:  # QK chunk h+1
                    for t in (h + 1, 6 + h + 1):
                        for q in range(NQ):
                            fillers.append(("qk", t, q))
                if h in (1, 2):  # V projection (wv lands ~22us)
                    for i in range(4 * (h - 1), 4 * (h - 1) + 4):
                        fillers.append(("v", i, 0))
                        fillers.append(("v", i, 1))
                if h >= 2:  # U for head h-2 (after V units at h=2)
                    for qc in range(NT):
                        fillers.append(("u", h - 2, qc))
                if h == 5:
                    fillers.append(("xfree",))
                if h >= 8:  # projA as late filler (needs attnT blocks 0..2)
                    for i in range(2 * (h - 8), 2 * (h - 8) + 2):
                        fillers.append(("pa", i, 0))
                        fillers.append(("pa", i, 1))

                def drain(k):
                    for _ in range(k):
                        if not fillers:
                            return
                        f = fillers.pop(0)
                        if f[0] == "v":
                            emit_v_unit(f[1], f[2])
                        elif f[0] == "qk":
                            emit_qk_unit(f[1], f[2])
                        elif f[0] == "u":
                            emit_u_unit(f[1], f[2])
                            if f[1] == 5:
                                # heads 0..5 done for this q-chunk:
                                # transpose attnT blocks 0..2
                                emit_transp(f[2], 0)
                        elif f[0] == "pa":
                            emit_proj_unit(f[1], f[2], 0)
                        elif f[0] == "xfree":
                            xp2.release()

                per = (len(fillers) + NT - 1) // NT
                for kc in range(NT):
                    emit_s_unit(h, kc)
                    drain(per)
                drain(len(fillers))
                if h >= 2:
                    expS_of.pop(h - 2)

            # ---- tail: U(10), U(11) + attnT blocks 3..5 + projB + store ----
            for qc in range(NT):
                emit_u_unit(10, qc)
            for qc in range(NT):
                emit_u_unit(11, qc)
                emit_transp(qc, 1)
            for qc in range(NT):
                emit_proj_unit(qc, 0, 1)
                emit_proj_unit(qc, 1, 1)
                nc.sync.dma_start(
                    out=out_d[qc * 128:(qc + 1) * 128, :], in_=o_acc[qc])
            iz.release()
            ep.release()
    nc.compile()
    return nc


_NC_CACHE = None


def kernel(x, W_qkv, b_qkv, W_proj, b_proj):
    from concourse.bass_utils import run_bass_kernel_spmd

    global _NC_CACHE
    if _NC_CACHE is None:
        _NC_CACHE = _build()
    nc = _NC_CACHE

    import ml_dtypes
    F8NP = ml_dtypes.float8_e4m3

    x = np.ascontiguousarray(np.asarray(x, dtype=np.float32))
    W_qkv = np.ascontiguousarray(np.asarray(W_qkv, dtype=np.float32))
    b_qkv = np.ascontiguousarray(np.asarray(b_qkv, dtype=np.float32))
    W_proj = np.ascontiguousarray(np.asarray(W_proj, dtype=np.float32))
    b_proj = np.ascontiguousarray(np.asarray(b_proj, dtype=np.float32))

    # host-side split-fp8: W_qkv*1024 = wh + wl (+ dropped eps)
    w_s = W_qkv * np.float32(1024.0)
    w_h = w_s.astype(F8NP)
    w_l = (w_s - w_h.astype(np.float32)).astype(F8NP)
    w_h = np.ascontiguousarray(w_h)
    w_l = np.ascontiguousarray(w_l)

    in_maps = [
        {"x": x[b], "W_qkvh": w_h, "W_qkvl": w_l, "b_qkv": b_qkv,
         "W_proj": W_proj, "b_proj": b_proj}
        for b in range(B)
    ]
    res = run_bass_kernel_spmd(nc, in_maps, core_ids=list(range(B)))
    return np.stack([np.asarray(res.results[b]["out"]) for b in range(B)])


# revision 10
# speedup vs baseline: 1.2524x; 1.1022x over previous
"""Multi-head attention block on 8 Trainium2 NeuronCores.

Problem: B=8, N=1024, E=768, H=12, D=64 attention (QKV proj -> softmax(QK^T/8)V
-> output proj), fp32 I/O. Data parallel over batch: core b owns batch b.

v4 design (split-fp8 DoubleRow QKV + all-bf16 attention, host preprocessing):
  - Host precomputes transposed split-fp8 x (xh+xl ~= 32*x^T) and split-fp8
    W_qkv (wh+wl ~= 1024*W_qkv), bf16 W_proj / bias rows. All device loads are
    plain HWDGE DMAs (no casts, no SWDGE descriptor generation, no PE
    transposes of x).
  - QKV projection: 9 fp8 DoubleRow matmuls per psum tile ((xh+xl)@(wh+wl)
    with the xl@wl term dropped), 256-deep contraction pairs at 0.5 cyc/row.
    Psum carries 2^15 scale; Q/K evac rescales (tensor_scalar mult+add bias),
    V keeps the scale which cancels against the 2^15 ones-column in Z.
  - S^T[k,q] per head: two 512-wide bf16 matmuls into a [128,1024] psum
    (contraction d=64 at partition base (h%2)*64); exp on Act -> bf16 expS.
  - U restructured: stationary = expS chunk [128k,128q], moving = V [128k,65]
    (64 dims + 2^15 ones column) -> psum U^T[q,65] accumulated over k chunks;
    invZ = reciprocal of column 64 is a per-partition scalar; attn = U*invZ
    is one DVE tensor_scalar op. Halves U's PE rows vs the classic layout
    and kills the PE invZ broadcast.
  - attn rows (token-major) -> attnT (feature-major) via HWDGE XBAR DMA
    transposes (3 [128,128] bf16 blocks per DMA, zero PE cost).
  - Output proj split: attnT blocks 0..2 projected during late attention as
    PE filler; blocks 3..5 in the tail, accumulated into the same SBUF tile.
  - Emission interleaves S psum fills with QK/V/U/proj filler units so the
    Act engine (exp is ~100us of work, the secondary wall) starves as little
    as possible while PE (the primary wall) stays busy.
"""
import numpy as np

B, N, E, H, D = 8, 1024, 768, 12, 64
SCALE = D ** -0.5
NT = N // 128   # token chunks (8)
NE = E // 128   # embed chunks (6)
NQ = N // 512   # moving-dim tiles (2)
NFS = [(0, 512), (512, 256)]  # free-dim split of E for matmuls
PROJ_SPLIT = 3  # attnT blocks 0..2 in projA (during attention), 3..5 in tail


def _build():
    import concourse.bacc as bacc
    import concourse.mybir as mybir
    import concourse.tile as tile

    F32 = mybir.dt.float32
    BF16 = mybir.dt.bfloat16
    F8 = mybir.dt.float8e4
    EXP = mybir.ActivationFunctionType.Exp
    DR = mybir.MatmulPerfMode.DoubleRow
    MUL = mybir.AluOpType.mult
    ADD = mybir.AluOpType.add

    nc = bacc.Bacc("TRN2", target_bir_lowering=False)
    xh_d = nc.declare_dram_parameter("xh", [E, N], F8, isOutput=False)
    xl_d = nc.declare_dram_parameter("xl", [E, N], F8, isOutput=False)
    wqkvh_d = nc.declare_dram_parameter("W_qkvh", [E, 3 * E], F8, isOutput=False)
    wqkvl_d = nc.declare_dram_parameter("W_qkvl", [E, 3 * E], F8, isOutput=False)
    bqkv_d = nc.declare_dram_parameter("b_qkv", [3 * E], F32, isOutput=False)
    bv_d = nc.declare_dram_parameter("b_v", [E], BF16, isOutput=False)
    wproj_d = nc.declare_dram_parameter("W_projb", [E, E], BF16, isOutput=False)
    bp_d = nc.declare_dram_parameter("b_pb", [E], BF16, isOutput=False)
    out_d = nc.declare_dram_parameter("out", [N, E], F32, isOutput=True)

    with tile.TileContext(nc) as tc:
        with (
            tc.tile_pool(name="const", bufs=1) as cp,
            tc.tile_pool(name="main", bufs=1) as qp,
            tc.tile_pool(name="psum", bufs=1, space="PSUM") as ps,
        ):
            # ---- constants ----
            ones1 = cp.tile([1, 128], BF16)
            nc.vector.memset(ones1, 1.0)
            ones32k = cp.tile([1, 128], BF16)
            nc.vector.memset(ones32k, 32768.0)
            bqc = cp.tile([128, 12], F32)   # column fc = b_qkv[128fc:128(fc+1)]

            # ---- long-lived tensors ----
            qT = [qp.tile([128, N], BF16, name=f"qT{c}", tag=f"qT{c}")
                  for c in range(6)]
            kT = [qp.tile([128, N], BF16, name=f"kT{c}", tag=f"kT{c}")
                  for c in range(6)]
            vS = [qp.tile([128, 65 * H], BF16, name=f"vS{i}", tag=f"vS{i}")
                  for i in range(NT)]
            attnS = [qp.tile([128, E], BF16, name=f"atS{i}", tag=f"atS{i}")
                     for i in range(NT)]
            attnT = qp.tile([128, NE * N], BF16)  # [128, (c, 1024)]
            attnTv = attnT.rearrange("p (c n) -> p c n", n=N)
            wp = [qp.tile([128, E], BF16, name=f"wp{c}", tag=f"wp{c}")
                  for c in range(6)]
            bv_bc = qp.tile([128, E], F32)
            bp_bc = qp.tile([128, E], F32)
            bv_row = qp.tile([1, E], BF16)
            bp_row = qp.tile([1, E], BF16)
            o_acc = [qp.tile([128, E], F32, name=f"oa{i}", tag=f"oa{i}")
                     for i in range(NT)]

            # expS pool: [128, N] bf16 tiles; 3 heads alive (lag 2)
            ep = tc.alloc_tile_pool(name="exp", bufs=1)
            iz = tc.alloc_tile_pool(name="iz", bufs=1)

            # scoped pool: x / W_qkv fp8 tiles, released once QKV is done
            xp = tc.alloc_tile_pool(name="xw", bufs=1)
            xH = xp.tile([128, NE * N], F8)   # [128, (j, 1024 tok)] = 32*x^T
            xL = xp.tile([128, NE * N], F8)
            xHv = xH.rearrange("p (j n) -> p j n", n=N)
            xLv = xL.rearrange("p (j n) -> p j n", n=N)
            # weight pair tiles: pair p = W rows 256p..256p+256 as [128, 2, f]
            wqkh = [xp.tile([128, 2 * 1536], F8, name=f"wqh{p}", tag=f"wqh{p}")
                    for p in range(3)]
            wqkl = [xp.tile([128, 2 * 1536], F8, name=f"wql{p}", tag=f"wql{p}")
                    for p in range(3)]
            wvh = [xp.tile([128, 2 * E], F8, name=f"wvh{p}", tag=f"wvh{p}")
                   for p in range(3)]
            wvl = [xp.tile([128, 2 * E], F8, name=f"wvl{p}", tag=f"wvl{p}")
                   for p in range(3)]

            # ---- DMAs (all HWDGE, no casts), in need order ----
            for j in range(NE):
                nc.sync.dma_start(
                    out=xHv[:, j, :], in_=xh_d[j * 128:(j + 1) * 128, :])
                nc.sync.dma_start(
                    out=xLv[:, j, :], in_=xl_d[j * 128:(j + 1) * 128, :])
            for p in range(3):
                for w_t, w_d in ((wqkh[p], wqkvh_d), (wqkl[p], wqkvl_d)):
                    nc.sync.dma_start(
                        out=w_t.rearrange("p (t f) -> p t f", t=2),
                        in_=w_d[256 * p:256 * (p + 1), 0:1536].rearrange(
                            "(t k) f -> k t f", t=2))
            nc.sync.dma_start(
                out=bqc, in_=bqkv_d[0:1536].rearrange("(f p) -> p f", p=128))
            nc.sync.dma_start(
                out=bv_row, in_=bv_d[:].rearrange("(o f) -> o f", o=1))
            nc.sync.dma_start(
                out=bp_row, in_=bp_d[:].rearrange("(o f) -> o f", o=1))
            for p in range(3):
                for w_t, w_d in ((wvh[p], wqkvh_d), (wvl[p], wqkvl_d)):
                    nc.sync.dma_start(
                        out=w_t.rearrange("p (t f) -> p t f", t=2),
                        in_=w_d[256 * p:256 * (p + 1), 1536:].rearrange(
                            "(t k) f -> k t f", t=2))
            for c in range(6):
                nc.sync.dma_start(
                    out=wp[c], in_=wproj_d[c * 128:(c + 1) * 128, :])

            def emit_prelude():
                for nf, (f0, fw) in enumerate(NFS):
                    pbv = ps.tile([128, 512], F32, name=f"pbv{nf}", tag="mm",
                                  bufs=2)
                    nc.tensor.matmul(pbv[:, :fw], ones32k,
                                     bv_row[:, f0:f0 + fw],
                                     start=True, stop=True)
                    nc.vector.tensor_copy(bv_bc[:, f0:f0 + fw], pbv[:, :fw])
                    pbp = ps.tile([128, 512], F32, name=f"pbp{nf}", tag="mm",
                                  bufs=2)
                    nc.tensor.matmul(pbp[:, :fw], ones1, bp_row[:, f0:f0 + fw],
                                     start=True, stop=True)
                    nc.vector.tensor_copy(bp_bc[:, f0:f0 + fw], pbp[:, :fw])
                for i in range(NT):
                    nc.vector.memset(
                        vS[i].rearrange("p (h c) -> p h c", c=65)[:, :, 64:65],
                        32768.0)

            # ================= emission units =================
            def emit_qk_unit(t, q):
                """One (feature-tile, 512-token-half) of Q or K projection.
                Split-fp8 DoubleRow: (xh+xl)@(wh+wl), xl@wl dropped."""
                dst = qT[t] if t < 6 else kT[t - 6]
                wcol0 = t * 128
                pq = ps.tile([128, 512], F32, name=f"pq{t}_{q}", tag="mm",
                             bufs=2)
                terms = [(wqkh, xHv), (wqkh, xLv), (wqkl, xHv)]
                nmm = 3 * len(terms)
                mi = 0
                for p in range(3):
                    for w_t, x_v in terms:
                        wv3 = w_t[p].rearrange("p (t f) -> p t f", t=2)
                        nc.tensor.matmul(
                            pq,
                            wv3[:, :, wcol0:wcol0 + 128],
                            x_v[:, 2 * p:2 * p + 2, q * 512:(q + 1) * 512],
                            start=(mi == 0), stop=(mi == nmm - 1),
                            perf_mode=DR)
                        mi += 1
                nc.vector.tensor_scalar(
                    out=dst[:, q * 512:(q + 1) * 512], in0=pq,
                    scalar1=float(2.0 ** -15), scalar2=bqc[:, t:t + 1],
                    op0=MUL, op1=ADD)

            def emit_v_unit(i, nf):
                """One (token-chunk, free-half) of the V projection."""
                f0, fw = NFS[nf]
                pv = ps.tile([128, 512], F32, name=f"pv{i}_{nf}", tag="mm",
                             bufs=2)
                terms = [(xHv, wvh), (xHv, wvl), (xLv, wvh)]
                nmm = 3 * len(terms)
                mi = 0
                for p in range(3):
                    for x_v, w_t in terms:
                        wv3 = w_t[p].rearrange("p (t f) -> p t f", t=2)
                        nc.tensor.matmul(
                            pv[:, :fw],
                            x_v[:, 2 * p:2 * p + 2, i * 128:(i + 1) * 128],
                            wv3[:, :, f0:f0 + fw],
                            start=(mi == 0), stop=(mi == nmm - 1),
                            perf_mode=DR)
                        mi += 1
                nh, h0 = fw // D, f0 // D
                nc.vector.tensor_add(
                    vS[i].rearrange("p (h c) -> p h c", c=65)
                        [:, h0:h0 + nh, 0:64],
                    pv[:, :fw].rearrange("p (h d) -> p h d", d=D),
                    bv_bc[:, f0:f0 + fw].rearrange("p (h d) -> p h d", d=D))

            expS_of = {}

            def emit_s_unit(h, kc):
                """S^T[k-chunk, all q] for one head + exp -> bf16 expS."""
                c, r0 = h // 2, (h % 2) * 64
                if kc == 0:
                    expS_of[h] = [
                        ep.tile([128, N], BF16, name=f"eS{h}_{k2}",
                                tag="expS", bufs=24)
                        for k2 in range(NT)]
                pss = ps.tile([128, N], F32, name=f"ps{h}_{kc}", tag="s2",
                              bufs=2)
                for q in range(NQ):
                    nc.tensor.matmul(
                        pss[:, q * 512:(q + 1) * 512],
                        kT[c][r0:r0 + 64, kc * 128:(kc + 1) * 128],
                        qT[c][r0:r0 + 64, q * 512:(q + 1) * 512],
                        start=True, stop=True)
                nc.scalar.activation(expS_of[h][kc], pss, EXP,
                                     scale=float(SCALE))

            def emit_u_unit(h, qc):
                """U^T[q-chunk, 65] for one head; normalize into attnS."""
                expS = expS_of[h]
                pu = ps.tile([128, 512], F32, name=f"pu{h}_{qc}", tag="u",
                             bufs=2)
                for kc in range(NT):
                    nc.tensor.matmul(
                        pu[:, 0:65],
                        expS[kc][:, qc * 128:(qc + 1) * 128],
                        vS[kc][:, h * 65:h * 65 + 65],
                        start=(kc == 0), stop=(kc == NT - 1))
                invz = iz.tile([128, 1], F32, name=f"iv{h}_{qc}", tag="iz",
                               bufs=3)
                nc.vector.reciprocal(invz, pu[:, 64:65])
                nc.vector.tensor_scalar_mul(
                    attnS[qc][:, h * D:(h + 1) * D], pu[:, 0:64], invz)

            def emit_transp(qc, half):
                """DMA-transpose 3 [128,128] bf16 blocks into attnT."""
                c0 = half * PROJ_SPLIT
                nc.sync.dma_start_transpose(
                    attnTv[:, c0:c0 + PROJ_SPLIT, qc * 128:(qc + 1) * 128],
                    attnS[qc][:, c0 * 128:(c0 + PROJ_SPLIT) * 128])

            def emit_proj_unit(i, nf, phase):
                """Half-row of output proj; phase 0 = blocks 0..2 (+bias),
                phase 1 = blocks 3..5 (+accumulate into o_acc) then store."""
                f0, fw = NFS[nf]
                cs = range(PROJ_SPLIT) if phase == 0 else range(PROJ_SPLIT, 6)
                po = ps.tile([128, 512], F32, name=f"po{i}_{nf}_{phase}",
                             tag="mm", bufs=2)
                for ci, c in enumerate(cs):
                    nc.tensor.matmul(
                        po[:, :fw],
                        attnTv[:, c, i * 128:(i + 1) * 128],
                        wp[c][:, f0:f0 + fw],
                        start=(ci == 0), stop=(ci == len(cs) - 1))
                if phase == 0:
                    nc.vector.tensor_add(
                        o_acc[i][:, f0:f0 + fw], po[:, :fw],
                        bp_bc[:, f0:f0 + fw])
                else:
                    nc.vector.tensor_add(
                        o_acc[i][:, f0:f0 + fw], po[:, :fw],
                        o_acc[i][:, f0:f0 + fw])

            # ================= schedule =================
            for t in (0, 6):
                for q in range(NQ):
                    emit_qk_unit(t, q)
            emit_prelude()

            for h in range(H):
                c = h // 2
                fillers = []
                if h < 10:  # QK chunk c+1: 2 units during each of h=2c, 2c+1
                    t0 = (c + 1, 6 + c + 1)
                    if h % 2 == 0:
                        fillers.append(("qk", t0[0], 0))
                        fillers.append(("qk", t0[1], 0))
                    else:
                        fillers.append(("qk", t0[0], 1))
                        fillers.append(("qk", t0[1], 1))
                if h in (1, 2):  # V projection (wv lands ~18us)
                    for i in range(4 * (h - 1), 4 * (h - 1) + 4):
                        fillers.append(("v", i, 0))
                        fillers.append(("v", i, 1))
                if h >= 2:  # U for head h-2 (after V units at h=2)
                    for qc in range(NT):
                        fillers.append(("u", h - 2, qc))
                if h == 10:
                    fillers.append(("xfree",))
                if h >= 8:  # projA as late filler (needs attnT blocks 0..2)
                    for i in range(2 * (h - 8), 2 * (h - 8) + 2):
                        fillers.append(("pa", i, 0))
                        fillers.append(("pa", i, 1))
                if h == 11:  # U(10) late in head 11 (exp(10) done by then)
                    for qc in range(NT):
                        fillers.append(("u", 10, qc))

                def drain(k):
                    for _ in range(k):
                        if not fillers:
                            return
                        f = fillers.pop(0)
                        if f[0] == "v":
                            emit_v_unit(f[1], f[2])
                        elif f[0] == "qk":
                            emit_qk_unit(f[1], f[2])
                        elif f[0] == "u":
                            emit_u_unit(f[1], f[2])
                            if f[1] == 5:
                                emit_transp(f[2], 0)
                        elif f[0] == "pa":
                            emit_proj_unit(f[1], f[2], 0)
                        elif f[0] == "xfree":
                            xp.release()

                per = (len(fillers) + NT - 1) // NT
                for kc in range(NT):
                    emit_s_unit(h, kc)
                    drain(per)
                drain(len(fillers))

            # ---- tail: U(11) + attnT blocks 3..5 + projB + store ----
            for qc in range(NT):
                emit_u_unit(11, qc)
                emit_transp(qc, 1)
            for qc in range(NT):
                emit_proj_unit(qc, 0, 1)
                emit_proj_unit(qc, 1, 1)
                nc.sync.dma_start(
                    out=out_d[qc * 128:(qc + 1) * 128, :], in_=o_acc[qc])
            iz.release()
            ep.release()
    nc.compile()
    return nc


_NC_CACHE = None


def kernel(x, W_qkv, b_qkv, W_proj, b_proj):
    from concourse.bass_utils import run_bass_kernel_spmd
    import ml_dtypes

    F8NP = ml_dtypes.float8_e4m3
    BF16NP = ml_dtypes.bfloat16

    global _NC_CACHE
    if _NC_CACHE is None:
        _NC_CACHE = _build()
    nc = _NC_CACHE

    x = np.asarray(x, dtype=np.float32)
    W_qkv = np.asarray(W_qkv, dtype=np.float32)
    b_qkv = np.ascontiguousarray(np.asarray(b_qkv, dtype=np.float32))
    W_proj = np.asarray(W_proj, dtype=np.float32)
    b_proj = np.asarray(b_proj, dtype=np.float32)

    # host-side preprocessing: transposed split-fp8 x, split-fp8 W_qkv,
    # bf16 W_proj and bias rows
    def split8(a):
        hi = a.astype(F8NP)
        lo = (a - hi.astype(np.float32)).astype(F8NP)
        return np.ascontiguousarray(hi), np.ascontiguousarray(lo)

    w_h, w_l = split8(W_qkv * np.float32(1024.0))
    xt32 = np.ascontiguousarray(np.swapaxes(x, 1, 2)) * np.float32(32.0)
    xsplit = [split8(xt32[b]) for b in range(B)]
    wp_b = np.ascontiguousarray(W_proj.astype(BF16NP))
    bv_b = np.ascontiguousarray(b_qkv[2 * E:].astype(BF16NP))
    bp_b = np.ascontiguousarray(b_proj.astype(BF16NP))

    in_maps = [
        {"xh": xsplit[b][0], "xl": xsplit[b][1],
         "W_qkvh": w_h, "W_qkvl": w_l, "b_qkv": b_qkv,
         "b_v": bv_b, "W_projb": wp_b, "b_pb": bp_b}
        for b in range(B)
    ]
    res = run_bass_kernel_spmd(nc, in_maps, core_ids=list(range(B)))
    return np.stack([np.asarray(res.results[b]["out"]) for b in range(B)])


# revision 25
# speedup vs baseline: 1.3021x; 1.0398x over previous
"""Multi-head attention block on 8 Trainium2 NeuronCores.

Problem: B=8, N=1024, E=768, H=12, D=64 attention (QKV proj -> softmax(QK^T/8)V
-> output proj), fp32 I/O. Data parallel over batch: core b owns batch b.

v4 design (split-fp8 DoubleRow QKV + all-bf16 attention, host preprocessing):
  - Host precomputes transposed split-fp8 x (xh+xl ~= 32*x^T) and split-fp8
    W_qkv (wh+wl ~= 1024*W_qkv), bf16 W_proj / bias rows. All device loads are
    plain HWDGE DMAs (no casts, no SWDGE descriptor generation, no PE
    transposes of x).
  - QKV projection: 9 fp8 DoubleRow matmuls per psum tile ((xh+xl)@(wh+wl)
    with the xl@wl term dropped), 256-deep contraction pairs at 0.5 cyc/row.
    Psum carries 2^15 scale; Q/K evac rescales (tensor_scalar mult+add bias),
    V keeps the scale which cancels against the 2^15 ones-column in Z.
  - S^T[k,q] per head: two 512-wide bf16 matmuls into a [128,1024] psum
    (contraction d=64 at partition base (h%2)*64); exp on Act -> bf16 expS.
  - U restructured: stationary = expS chunk [128k,128q], moving = V [128k,65]
    (64 dims + 2^15 ones column) -> psum U^T[q,65] accumulated over k chunks;
    invZ = reciprocal of column 64 is a per-partition scalar; attn = U*invZ
    is one DVE tensor_scalar op. Halves U's PE rows vs the classic layout
    and kills the PE invZ broadcast.
  - attn rows (token-major) -> attnT (feature-major) via HWDGE XBAR DMA
    transposes (3 [128,128] bf16 blocks per DMA, zero PE cost).
  - Output proj split: attnT blocks 0..2 projected during late attention as
    PE filler; blocks 3..5 in the tail, accumulated into the same SBUF tile.
  - Emission interleaves S psum fills with QK/V/U/proj filler units so the
    Act engine (exp is ~100us of work, the secondary wall) starves as little
    as possible while PE (the primary wall) stays busy.
"""
import numpy as np

B, N, E, H, D = 8, 1024, 768, 12, 64
SCALE = D ** -0.5
NT = N // 128   # token chunks (8)
NE = E // 128   # embed chunks (6)
NQ = N // 512   # moving-dim tiles (2)
NFS = [(0, 512), (512, 256)]  # free-dim split of E for matmuls
PROJ_SPLIT = 3  # attnT blocks 0..2 in projA (during attention), 3..5 in tail


def _build():
    import concourse.bacc as bacc
    import concourse.mybir as mybir
    import concourse.tile as tile

    F32 = mybir.dt.float32
    BF16 = mybir.dt.bfloat16
    F8 = mybir.dt.float8e4
    EXP = mybir.ActivationFunctionType.Exp
    DR = mybir.MatmulPerfMode.DoubleRow
    MUL = mybir.AluOpType.mult
    ADD = mybir.AluOpType.add

    nc = bacc.Bacc("TRN2", target_bir_lowering=False)
    xh_d = nc.declare_dram_parameter("xh", [E, N], F8, isOutput=False)
    xl_d = nc.declare_dram_parameter("xl", [E, N], F8, isOutput=False)
    wqkvh_d = nc.declare_dram_parameter("W_qkvh", [E, 3 * E], F8, isOutput=False)
    wqkvl_d = nc.declare_dram_parameter("W_qkvl", [E, 3 * E], F8, isOutput=False)
    bqkv_d = nc.declare_dram_parameter("b_qkv", [3 * E], F32, isOutput=False)
    bv_d = nc.declare_dram_parameter("b_v", [E], BF16, isOutput=False)
    wprojh_d = nc.declare_dram_parameter("W_projh", [E, E], F8, isOutput=False)
    wprojl_d = nc.declare_dram_parameter("W_projl", [E, E], F8, isOutput=False)
    wpb45_d = nc.declare_dram_parameter("W_pb45", [256, E], BF16, isOutput=False)
    bp_d = nc.declare_dram_parameter("b_pb", [E], BF16, isOutput=False)
    out_d = nc.declare_dram_parameter("out", [N, E], F32, isOutput=True)

    with tile.TileContext(nc) as tc:
        with (
            tc.tile_pool(name="const", bufs=1) as cp,
            tc.tile_pool(name="main", bufs=1) as qp,
            tc.tile_pool(name="psum", bufs=1, space="PSUM") as ps,
        ):
            # ---- constants ----
            ones1 = cp.tile([1, 128], BF16)
            nc.vector.memset(ones1, 1.0)
            ones32k = cp.tile([1, 128], BF16)
            nc.vector.memset(ones32k, 32768.0)
            bqc = cp.tile([128, 12], F32)   # column fc = b_qkv[128fc:128(fc+1)]

            # ---- long-lived tensors ----
            qT = [qp.tile([128, N], BF16, name=f"qT{c}", tag=f"qT{c}")
                  for c in range(6)]
            kT = [qp.tile([128, N], BF16, name=f"kT{c}", tag=f"kT{c}")
                  for c in range(6)]
            vS = [qp.tile([128, 65 * H], BF16, name=f"vS{i}", tag=f"vS{i}")
                  for i in range(NT)]
            attnS = [qp.tile([128, E], BF16, name=f"atS{i}", tag=f"atS{i}")
                     for i in range(NT)]
            attnT = qp.tile([128, NE * N], BF16)  # [128, (c, 1024)] = 32*attn^T
            attnTv = attnT.rearrange("p (c n) -> p c n", n=N)
            attnTh = qp.tile([128, NE * N], F8)
            attnThv = attnTh.rearrange("p (c n) -> p c n", n=N)
            attnTl = qp.tile([128, NE * N], F8)
            attnTlv = attnTl.rearrange("p (c n) -> p c n", n=N)
            # W_proj pair tiles: pair pp = rows 256pp..256pp+256 as [128, 2, E]
            wphB = qp.tile([128, 2 * 2 * E], F8)
            wplB = qp.tile([128, 2 * 2 * E], F8)
            wphv = wphB.rearrange("p (pp t f) -> p pp t f", t=2, f=E)
            wplv = wplB.rearrange("p (pp t f) -> p pp t f", t=2, f=E)
            wpb45 = qp.tile([128, 2 * E], BF16)
            wpb45v = wpb45.rearrange("p (c f) -> p c f", f=E)
            bv_bc = qp.tile([128, E], F32)
            bp_bc = qp.tile([128, E], F32)
            bv_row = qp.tile([1, E], BF16)
            bp_row = qp.tile([1, E], BF16)
            o_acc = [qp.tile([128, E], F32, name=f"oa{i}", tag=f"oa{i}")
                     for i in range(NT)]

            # expS pool: [128, N] bf16 tiles; 3 heads alive (lag 2)
            ep = tc.alloc_tile_pool(name="exp", bufs=1)
            iz = tc.alloc_tile_pool(name="iz", bufs=1)

            # scoped pool: x / W_qkv fp8 tiles, released once QKV is done
            xp = tc.alloc_tile_pool(name="xw", bufs=1)
            xH = xp.tile([128, NE * N], F8)   # [128, (j, 1024 tok)] = 32*x^T
            xL = xp.tile([128, NE * N], F8)
            xHv = xH.rearrange("p (j n) -> p j n", n=N)
            xLv = xL.rearrange("p (j n) -> p j n", n=N)
            # weight pair big tiles: [128, (p, t, f)] with pair p = W rows
            # 256p..256p+256 split as 2 k-subtiles t
            wqkhB = xp.tile([128, 3 * 2 * 1536], F8)
            wqklB = xp.tile([128, 3 * 2 * 1536], F8)
            wqkhv = wqkhB.rearrange("p (pp t f) -> p pp t f", t=2, f=1536)
            wqklv = wqklB.rearrange("p (pp t f) -> p pp t f", t=2, f=1536)
            wvhB = xp.tile([128, 3 * 2 * E], F8)
            wvlB = xp.tile([128, 3 * 2 * E], F8)
            wvhv = wvhB.rearrange("p (pp t f) -> p pp t f", t=2, f=E)
            wvlv = wvlB.rearrange("p (pp t f) -> p pp t f", t=2, f=E)

            # ---- DMAs (all HWDGE, no casts): few big transfers, with two
            # small priority slices so the first S unit starts early ----
            def wqk_slice(wview, w_d, c0, cw):
                nc.sync.dma_start(
                    out=wview[:, :, :, c0:c0 + cw],
                    in_=w_d[0:768, c0:c0 + cw].rearrange(
                        "(pp t k) f -> k pp t f", t=2, k=128))

            # 1. t=0 / t=6 weight columns (gate the first S unit)
            wqk_slice(wqkhv, wqkvh_d, 0, 128)
            wqk_slice(wqkhv, wqkvh_d, 768, 128)
            wqk_slice(wqklv, wqkvl_d, 0, 128)
            wqk_slice(wqklv, wqkvl_d, 768, 128)
            nc.sync.dma_start(
                out=bqc, in_=bqkv_d[0:1536].rearrange("(f p) -> p f", p=128))
            # 2. x token-half 0, then half 1
            for half in range(2):
                t0 = half * 512
                nc.sync.dma_start(
                    out=xHv[:, :, t0:t0 + 512],
                    in_=xh_d[:, t0:t0 + 512].rearrange(
                        "(j k) n -> k j n", k=128))
                nc.sync.dma_start(
                    out=xLv[:, :, t0:t0 + 512],
                    in_=xl_d[:, t0:t0 + 512].rearrange(
                        "(j k) n -> k j n", k=128))
            nc.sync.dma_start(
                out=bv_row, in_=bv_d[:].rearrange("(o f) -> o f", o=1))
            nc.sync.dma_start(
                out=bp_row, in_=bp_d[:].rearrange("(o f) -> o f", o=1))
            # 3. remaining wqk columns
            wqk_slice(wqkhv, wqkvh_d, 128, 640)
            wqk_slice(wqklv, wqkvl_d, 128, 640)
            wqk_slice(wqkhv, wqkvh_d, 896, 640)
            wqk_slice(wqklv, wqkvl_d, 896, 640)
            # 4. V weights, bias rows, proj weights
            for w_t, w_d in ((wvhB, wqkvh_d), (wvlB, wqkvl_d)):
                nc.sync.dma_start(
                    out=w_t.rearrange("p (pp t f) -> p pp t f", t=2, f=E),
                    in_=w_d[0:768, 1536:].rearrange(
                        "(pp t k) f -> k pp t f", t=2, k=128))
            for w_t, w_d in ((wphB, wprojh_d), (wplB, wprojl_d)):
                nc.sync.dma_start(
                    out=w_t.rearrange("p (pp t f) -> p pp t f", t=2, f=E),
                    in_=w_d[0:512, :].rearrange(
                        "(pp t k) f -> k pp t f", t=2, k=128))
            nc.sync.dma_start(
                out=wpb45.rearrange("p (c f) -> p c f", f=E),
                in_=wpb45_d[:].rearrange("(c k) f -> k c f", k=128))

            def emit_prelude():
                for nf, (f0, fw) in enumerate(NFS):
                    pbv = ps.tile([128, 512], F32, name=f"pbv{nf}", tag="mm",
                                  bufs=2)
                    nc.tensor.matmul(pbv[:, :fw], ones32k,
                                     bv_row[:, f0:f0 + fw],
                                     start=True, stop=True)
                    nc.vector.tensor_copy(bv_bc[:, f0:f0 + fw], pbv[:, :fw])
                    pbp = ps.tile([128, 512], F32, name=f"pbp{nf}", tag="mm",
                                  bufs=2)
                    nc.tensor.matmul(pbp[:, :fw], ones32k,
                                     bp_row[:, f0:f0 + fw],
                                     start=True, stop=True)
                    nc.vector.tensor_copy(bp_bc[:, f0:f0 + fw], pbp[:, :fw])
                for i in range(NT):
                    nc.vector.memset(
                        vS[i].rearrange("p (h c) -> p h c", c=65)[:, :, 64:65],
                        32768.0)

            # ================= emission units =================
            def emit_qk_unit(t, q, evac_act=False):
                """One (feature-tile, 512-token-half) of Q or K projection.
                Split-fp8 DoubleRow: (xh+xl)@(wh+wl), xl@wl dropped."""
                dst = qT[t] if t < 6 else kT[t - 6]
                wcol0 = t * 128
                pq = ps.tile([128, 512], F32, name=f"pq{t}_{q}", tag="mm",
                             bufs=2)
                terms = [(wqkhv, xHv), (wqkhv, xLv), (wqklv, xHv)]
                nmm = 3 * len(terms)
                mi = 0
                for p in range(3):
                    for w_v, x_v in terms:
                        nc.tensor.matmul(
                            pq,
                            w_v[:, p, :, wcol0:wcol0 + 128],
                            x_v[:, 2 * p:2 * p + 2, q * 512:(q + 1) * 512],
                            start=(mi == 0), stop=(mi == nmm - 1),
                            perf_mode=DR)
                        mi += 1
                if evac_act:
                    nc.scalar.activation(
                        dst[:, q * 512:(q + 1) * 512], pq,
                        mybir.ActivationFunctionType.Identity,
                        bias=bqc[:, t:t + 1], scale=float(2.0 ** -15))
                else:
                    nc.vector.tensor_scalar(
                        out=dst[:, q * 512:(q + 1) * 512], in0=pq,
                        scalar1=float(2.0 ** -15), scalar2=bqc[:, t:t + 1],
                        op0=MUL, op1=ADD)

            def emit_v_unit(i, nf):
                """One (token-chunk, free-half) of the V projection."""
                f0, fw = NFS[nf]
                pv = ps.tile([128, 512], F32, name=f"pv{i}_{nf}", tag="mm",
                             bufs=2)
                terms = [(xHv, wvhv), (xHv, wvlv), (xLv, wvhv)]
                nmm = 3 * len(terms)
                mi = 0
                for p in range(3):
                    for x_v, w_v in terms:
                        nc.tensor.matmul(
                            pv[:, :fw],
                            x_v[:, 2 * p:2 * p + 2, i * 128:(i + 1) * 128],
                            w_v[:, p, :, f0:f0 + fw],
                            start=(mi == 0), stop=(mi == nmm - 1),
                            perf_mode=DR)
                        mi += 1
                nh, h0 = fw // D, f0 // D
                nc.vector.tensor_add(
                    vS[i].rearrange("p (h c) -> p h c", c=65)
                        [:, h0:h0 + nh, 0:64],
                    pv[:, :fw].rearrange("p (h d) -> p h d", d=D),
                    bv_bc[:, f0:f0 + fw].rearrange("p (h d) -> p h d", d=D))

            expS_of = {}

            def emit_s_unit(h, kc):
                """S^T[k-chunk, all q] for one head + exp -> bf16 expS."""
                c, r0 = h // 2, (h % 2) * 64
                if kc == 0:
                    expS_of[h] = [
                        ep.tile([128, N], BF16, name=f"eS{h}_{k2}",
                                tag="expS", bufs=24)
                        for k2 in range(NT)]
                pss = ps.tile([128, N], F32, name=f"ps{h}_{kc}", tag="s2",
                              bufs=2)
                for q in range(NQ):
                    nc.tensor.matmul(
                        pss[:, q * 512:(q + 1) * 512],
                        kT[c][r0:r0 + 64, kc * 128:(kc + 1) * 128],
                        qT[c][r0:r0 + 64, q * 512:(q + 1) * 512],
                        start=True, stop=True)
                nc.scalar.activation(expS_of[h][kc], pss, EXP,
                                     scale=float(SCALE))

            def emit_u_unit(h, qc):
                """U^T[q-chunk, 65] for one head; normalize into attnS."""
                expS = expS_of[h]
                pu = ps.tile([128, 512], F32, name=f"pu{h}_{qc}", tag="u",
                             bufs=2)
                for kc in range(NT):
                    nc.tensor.matmul(
                        pu[:, 0:65],
                        expS[kc][:, qc * 128:(qc + 1) * 128],
                        vS[kc][:, h * 65:h * 65 + 65],
                        start=(kc == 0), stop=(kc == NT - 1))
                invz = iz.tile([128, 1], F32, name=f"iv{h}_{qc}", tag="iz",
                               bufs=3)
                nc.vector.reciprocal(invz, pu[:, 64:65])
                nc.vector.tensor_scalar(
                    out=attnS[qc][:, h * D:(h + 1) * D], in0=pu[:, 0:64],
                    scalar1=invz, scalar2=32.0, op0=MUL, op1=MUL)

            def emit_transp(qc, pp):
                """DMA-transpose one block pair into attnT (32*attn^T bf16),
                then split to fp8 hi/lo on the Pool engine."""
                c0 = 2 * pp
                t_bf = attnTv[:, c0:c0 + 2, qc * 128:(qc + 1) * 128]
                nc.sync.dma_start_transpose(
                    t_bf, attnS[qc][:, c0 * 128:(c0 + 2) * 128])
                if pp < 2:
                    t_hi = attnThv[:, c0:c0 + 2, qc * 128:(qc + 1) * 128]
                    nc.gpsimd.tensor_copy(t_hi, t_bf)
                    nc.gpsimd.tensor_sub(
                        attnTlv[:, c0:c0 + 2, qc * 128:(qc + 1) * 128],
                        t_bf, t_hi)

            def emit_proj_unit(i, nf, pp):
                """Half-row of output proj for block pair pp (split-fp8 DR).
                pp=0 adds the (2^15-scaled) bias; pp=1 accumulates; pp=2
                accumulates and rescales to the final f32 output."""
                f0, fw = NFS[nf]
                po = ps.tile([128, 512], F32, name=f"po{i}_{nf}_{pp}",
                             tag="mm", bufs=2)
                if pp < 2:
                    terms = [(attnThv, wphv), (attnThv, wplv), (attnTlv, wphv)]
                    for mi, (a_v, w_v) in enumerate(terms):
                        nc.tensor.matmul(
                            po[:, :fw],
                            a_v[:, 2 * pp:2 * pp + 2, i * 128:(i + 1) * 128],
                            w_v[:, pp, :, f0:f0 + fw],
                            start=(mi == 0), stop=(mi == len(terms) - 1),
                            perf_mode=DR)
                else:
                    for ci, cb in enumerate((4, 5)):
                        nc.tensor.matmul(
                            po[:, :fw],
                            attnTv[:, cb, i * 128:(i + 1) * 128],
                            wpb45v[:, cb - 4, f0:f0 + fw],
                            start=(ci == 0), stop=(ci == 1))
                if pp == 0:
                    nc.vector.tensor_add(
                        o_acc[i][:, f0:f0 + fw], po[:, :fw],
                        bp_bc[:, f0:f0 + fw])
                elif pp == 1:
                    nc.vector.tensor_add(
                        o_acc[i][:, f0:f0 + fw], po[:, :fw],
                        o_acc[i][:, f0:f0 + fw])
                else:
                    nc.vector.tensor_add(
                        o_acc[i][:, f0:f0 + fw], po[:, :fw],
                        o_acc[i][:, f0:f0 + fw])
                    nc.gpsimd.tensor_scalar_mul(
                        o_acc[i][:, f0:f0 + fw], o_acc[i][:, f0:f0 + fw],
                        float(2.0 ** -15))

            # ================= schedule =================
            for q in range(NQ):
                for t in (0, 6):
                    emit_qk_unit(t, q)

            for h in range(H):
                c = h // 2
                fillers = []
                if h == 0:
                    fillers.append(("prelude",))
                if h < 10:  # QK chunk c+1: 2 units during each of h=2c, 2c+1
                    t0 = (c + 1, 6 + c + 1)
                    if h % 2 == 0:
                        fillers.append(("qk", t0[0], 0))
                        fillers.append(("qk", t0[1], 0))
                    else:
                        fillers.append(("qk", t0[0], 1))
                        fillers.append(("qk", t0[1], 1))
                if h in (1, 2):  # V projection (wv lands ~18us)
                    for i in range(4 * (h - 1), 4 * (h - 1) + 4):
                        fillers.append(("v", i, 0))
                        fillers.append(("v", i, 1))
                if h >= 2:  # U for head h-2 (after V units at h=2)
                    for qc in range(NT):
                        fillers.append(("u", h - 2, qc))
                if h == 10:
                    fillers.append(("xfree",))
                if h in (6, 7):  # proj pair 0 (blocks 0,1; ready after U(3))
                    for i in range(4 * (h - 6), 4 * (h - 6) + 4):
                        fillers.append(("pa", i, 0, 0))
                        fillers.append(("pa", i, 1, 0))
                if h == 10:  # proj pair 1 (blocks 2,3; ready after U(7))
                    for i in range(NT):
                        fillers.append(("pa", i, 0, 1))
                        fillers.append(("pa", i, 1, 1))
                if h == 11:  # U(10) late in head 11 (exp(10) done by then)
                    for qc in range(NT):
                        fillers.append(("u", 10, qc))

                fillers.sort(key=lambda f: f[0] == "u")

                fillers.sort(key=lambda f: f[0] == "u")

                def drain(k):
                    for _ in range(k):
                        if not fillers:
                            return
                        f = fillers.pop(0)
                        if f[0] == "v":
                            emit_v_unit(f[1], f[2])
                        elif f[0] == "qk":
                            emit_qk_unit(f[1], f[2])
                        elif f[0] == "u":
                            emit_u_unit(f[1], f[2])
                            if f[1] == 3:
                                emit_transp(f[2], 0)
                            elif f[1] == 7:
                                emit_transp(f[2], 1)
                        elif f[0] == "pa":
                            emit_proj_unit(f[1], f[2], f[3])
                        elif f[0] == "xfree":
                            xp.release()
                        elif f[0] == "prelude":
                            emit_prelude()

                per = (len(fillers) + NT - 1) // NT
                for kc in range(NT):
                    emit_s_unit(h, kc)
                    drain(per)
                drain(len(fillers))

            # ---- tail: U(11) + attnT blocks 4,5 + proj pair 2, pipelined
            def emit_out(i):
                emit_proj_unit(i, 0, 2)
                emit_proj_unit(i, 1, 2)
                nc.sync.dma_start(
                    out=out_d[i * 128:(i + 1) * 128, :], in_=o_acc[i])

            for qc in range(NT):
                emit_u_unit(11, qc)
                emit_transp(qc, 2)
                if qc >= 1:
                    emit_out(qc - 1)
            emit_out(NT - 1)
            iz.release()
            ep.release()
    nc.compile()
    return nc


_NC_CACHE = None


def kernel(x, W_qkv, b_qkv, W_proj, b_proj):
    from concourse.bass_utils import run_bass_kernel_spmd
    import ml_dtypes

    F8NP = ml_dtypes.float8_e4m3
    BF16NP = ml_dtypes.bfloat16

    global _NC_CACHE
    if _NC_CACHE is None:
        _NC_CACHE = _build()
    nc = _NC_CACHE

    x = np.asarray(x, dtype=np.float32)
    W_qkv = np.asarray(W_qkv, dtype=np.float32)
    b_qkv = np.ascontiguousarray(np.asarray(b_qkv, dtype=np.float32))
    W_proj = np.asarray(W_proj, dtype=np.float32)
    b_proj = np.asarray(b_proj, dtype=np.float32)

    # host-side preprocessing: transposed split-fp8 x, split-fp8 W_qkv,
    # bf16 W_proj and bias rows
    def split8(a):
        hi = a.astype(F8NP)
        lo = (a - hi.astype(np.float32)).astype(F8NP)
        return np.ascontiguousarray(hi), np.ascontiguousarray(lo)

    w_h, w_l = split8(W_qkv * np.float32(1024.0))
    xt32 = np.ascontiguousarray(np.swapaxes(x, 1, 2)) * np.float32(32.0)
    xsplit = [split8(xt32[b]) for b in range(B)]
    wp_h, wp_l = split8(W_proj * np.float32(1024.0))
    wp45 = np.ascontiguousarray(
        (W_proj[512:, :] * np.float32(1024.0)).astype(BF16NP))
    bv_b = np.ascontiguousarray(b_qkv[2 * E:].astype(BF16NP))
    bp_b = np.ascontiguousarray(b_proj.astype(BF16NP))

    in_maps = [
        {"xh": xsplit[b][0], "xl": xsplit[b][1],
         "W_qkvh": w_h, "W_qkvl": w_l, "b_qkv": b_qkv,
         "b_v": bv_b, "W_projh": wp_h, "W_projl": wp_l, "W_pb45": wp45,
         "b_pb": bp_b}
        for b in range(B)
    ]
    res = run_bass_kernel_spmd(nc, in_maps, core_ids=list(range(B)))
    return np.stack([np.asarray(res.results[b]["out"]) for b in range(B)])


# revision 31
# speedup vs baseline: 1.3031x; 1.0008x over previous
"""Multi-head attention block on 8 Trainium2 NeuronCores.

Problem: B=8, N=1024, E=768, H=12, D=64 attention (QKV proj -> softmax(QK^T/8)V
-> output proj), fp32 I/O. Data parallel over batch: core b owns batch b.

v4 design (split-fp8 DoubleRow QKV + all-bf16 attention, host preprocessing):
  - Host precomputes transposed split-fp8 x (xh+xl ~= 32*x^T) and split-fp8
    W_qkv (wh+wl ~= 1024*W_qkv), bf16 W_proj / bias rows. All device loads are
    plain HWDGE DMAs (no casts, no SWDGE descriptor generation, no PE
    transposes of x).
  - QKV projection: 9 fp8 DoubleRow matmuls per psum tile ((xh+xl)@(wh+wl)
    with the xl@wl term dropped), 256-deep contraction pairs at 0.5 cyc/row.
    Psum carries 2^15 scale; Q/K evac rescales (tensor_scalar mult+add bias),
    V keeps the scale which cancels against the 2^15 ones-column in Z.
  - S^T[k,q] per head: two 512-wide bf16 matmuls into a [128,1024] psum
    (contraction d=64 at partition base (h%2)*64); exp on Act -> bf16 expS.
  - U restructured: stationary = expS chunk [128k,128q], moving = V [128k,65]
    (64 dims + 2^15 ones column) -> psum U^T[q,65] accumulated over k chunks;
    invZ = reciprocal of column 64 is a per-partition scalar; attn = U*invZ
    is one DVE tensor_scalar op. Halves U's PE rows vs the classic layout
    and kills the PE invZ broadcast.
  - attn rows (token-major) -> attnT (feature-major) via HWDGE XBAR DMA
    transposes (3 [128,128] bf16 blocks per DMA, zero PE cost).
  - Output proj split: attnT blocks 0..2 projected during late attention as
    PE filler; blocks 3..5 in the tail, accumulated into the same SBUF tile.
  - Emission interleaves S psum fills with QK/V/U/proj filler units so the
    Act engine (exp is ~100us of work, the secondary wall) starves as little
    as possible while PE (the primary wall) stays busy.
"""
import numpy as np

B, N, E, H, D = 8, 1024, 768, 12, 64
SCALE = D ** -0.5
NT = N // 128   # token chunks (8)
NE = E // 128   # embed chunks (6)
NQ = N // 512   # moving-dim tiles (2)
NFS = [(0, 512), (512, 256)]  # free-dim split of E for matmuls
PROJ_SPLIT = 3  # attnT blocks 0..2 in projA (during attention), 3..5 in tail


def _build():
    import concourse.bacc as bacc
    import concourse.mybir as mybir
    import concourse.tile as tile

    F32 = mybir.dt.float32
    BF16 = mybir.dt.bfloat16
    F8 = mybir.dt.float8e4
    EXP = mybir.ActivationFunctionType.Exp
    DR = mybir.MatmulPerfMode.DoubleRow
    MUL = mybir.AluOpType.mult
    ADD = mybir.AluOpType.add

    nc = bacc.Bacc("TRN2", target_bir_lowering=False)
    xh_d = nc.declare_dram_parameter("xh", [E, N], F8, isOutput=False)
    xl_d = nc.declare_dram_parameter("xl", [E, N], F8, isOutput=False)
    wqkvh_d = nc.declare_dram_parameter("W_qkvh", [E, 3 * E], F8, isOutput=False)
    wqkvl_d = nc.declare_dram_parameter("W_qkvl", [E, 3 * E], F8, isOutput=False)
    bqkv_d = nc.declare_dram_parameter("b_qkv", [3 * E], F32, isOutput=False)
    bv_d = nc.declare_dram_parameter("b_v", [E], BF16, isOutput=False)
    wprojh_d = nc.declare_dram_parameter("W_projh", [E, E], F8, isOutput=False)
    wprojl_d = nc.declare_dram_parameter("W_projl", [E, E], F8, isOutput=False)
    wpb45_d = nc.declare_dram_parameter("W_pb45", [256, E], BF16, isOutput=False)
    bp_d = nc.declare_dram_parameter("b_pb", [E], BF16, isOutput=False)
    out_d = nc.declare_dram_parameter("out", [N, E], F32, isOutput=True)

    with tile.TileContext(nc) as tc:
        with (
            tc.tile_pool(name="const", bufs=1) as cp,
            tc.tile_pool(name="main", bufs=1) as qp,
            tc.tile_pool(name="psum", bufs=1, space="PSUM") as ps,
        ):
            # ---- constants ----
            ones1 = cp.tile([1, 128], BF16)
            nc.vector.memset(ones1, 1.0)
            ones32k = cp.tile([1, 128], BF16)
            nc.vector.memset(ones32k, 32768.0)
            bqc = cp.tile([128, 12], F32)   # column fc = b_qkv[128fc:128(fc+1)]

            # ---- long-lived tensors ----
            qT = [qp.tile([128, N], BF16, name=f"qT{c}", tag=f"qT{c}")
                  for c in range(6)]
            kT = [qp.tile([128, N], BF16, name=f"kT{c}", tag=f"kT{c}")
                  for c in range(6)]
            vS = [qp.tile([128, 65 * H], BF16, name=f"vS{i}", tag=f"vS{i}")
                  for i in range(NT)]
            attnS = [qp.tile([128, E], BF16, name=f"atS{i}", tag=f"atS{i}")
                     for i in range(NT)]
            attnT = qp.tile([128, NE * N], BF16)  # [128, (c, 1024)] = 32*attn^T
            attnTv = attnT.rearrange("p (c n) -> p c n", n=N)
            attnTh = qp.tile([128, NE * N], F8)
            attnThv = attnTh.rearrange("p (c n) -> p c n", n=N)
            attnTl = qp.tile([128, NE * N], F8)
            attnTlv = attnTl.rearrange("p (c n) -> p c n", n=N)
            # W_proj pair tiles: pair pp = rows 256pp..256pp+256 as [128, 2, E]
            wphB = qp.tile([128, 2 * 2 * E], F8)
            wplB = qp.tile([128, 2 * 2 * E], F8)
            wphv = wphB.rearrange("p (pp t f) -> p pp t f", t=2, f=E)
            wplv = wplB.rearrange("p (pp t f) -> p pp t f", t=2, f=E)
            wpb45 = qp.tile([128, 2 * E], BF16)
            wpb45v = wpb45.rearrange("p (c f) -> p c f", f=E)
            bv_bc = qp.tile([128, E], F32)
            bp_bc = qp.tile([128, E], F32)
            bv_row = qp.tile([1, E], BF16)
            bp_row = qp.tile([1, E], BF16)
            o_acc = [qp.tile([128, E], F32, name=f"oa{i}", tag=f"oa{i}")
                     for i in range(NT)]

            # expS pool: [128, N] bf16 tiles; 3 heads alive (lag 2)
            ep = tc.alloc_tile_pool(name="exp", bufs=1)
            iz = tc.alloc_tile_pool(name="iz", bufs=1)

            # scoped pool: x / W_qkv fp8 tiles, released once QKV is done
            xp = tc.alloc_tile_pool(name="xw", bufs=1)
            xH = xp.tile([128, NE * N], F8)   # [128, (j, 1024 tok)] = 32*x^T
            xL = xp.tile([128, NE * N], F8)
            xHv = xH.rearrange("p (j n) -> p j n", n=N)
            xLv = xL.rearrange("p (j n) -> p j n", n=N)
            # weight pair big tiles: [128, (p, t, f)] with pair p = W rows
            # 256p..256p+256 split as 2 k-subtiles t
            wqkhB = xp.tile([128, 3 * 2 * 1536], F8)
            wqklB = xp.tile([128, 3 * 2 * 1536], F8)
            wqkhv = wqkhB.rearrange("p (pp t f) -> p pp t f", t=2, f=1536)
            wqklv = wqklB.rearrange("p (pp t f) -> p pp t f", t=2, f=1536)
            wvhB = xp.tile([128, 3 * 2 * E], F8)
            wvlB = xp.tile([128, 3 * 2 * E], F8)
            wvhv = wvhB.rearrange("p (pp t f) -> p pp t f", t=2, f=E)
            wvlv = wvlB.rearrange("p (pp t f) -> p pp t f", t=2, f=E)

            # ---- DMAs (all HWDGE, no casts): few big transfers, with two
            # small priority slices so the first S unit starts early ----
            def wqk_slice(wview, w_d, c0, cw):
                nc.sync.dma_start(
                    out=wview[:, :, :, c0:c0 + cw],
                    in_=w_d[0:768, c0:c0 + cw].rearrange(
                        "(pp t k) f -> k pp t f", t=2, k=128))

            # 1. t=0 / t=6 weight columns (gate the first S unit)
            wqk_slice(wqkhv, wqkvh_d, 0, 128)
            wqk_slice(wqkhv, wqkvh_d, 768, 128)
            wqk_slice(wqklv, wqkvl_d, 0, 128)
            wqk_slice(wqklv, wqkvl_d, 768, 128)
            nc.sync.dma_start(
                out=bqc, in_=bqkv_d[0:1536].rearrange("(f p) -> p f", p=128))
            # 2. x token-half 0, then half 1
            for half in range(2):
                t0 = half * 512
                nc.sync.dma_start(
                    out=xHv[:, :, t0:t0 + 512],
                    in_=xh_d[:, t0:t0 + 512].rearrange(
                        "(j k) n -> k j n", k=128))
                nc.sync.dma_start(
                    out=xLv[:, :, t0:t0 + 512],
                    in_=xl_d[:, t0:t0 + 512].rearrange(
                        "(j k) n -> k j n", k=128))
            nc.sync.dma_start(
                out=bv_row, in_=bv_d[:].rearrange("(o f) -> o f", o=1))
            nc.sync.dma_start(
                out=bp_row, in_=bp_d[:].rearrange("(o f) -> o f", o=1))
            # 3. remaining wqk columns
            wqk_slice(wqkhv, wqkvh_d, 128, 640)
            wqk_slice(wqklv, wqkvl_d, 128, 640)
            wqk_slice(wqkhv, wqkvh_d, 896, 640)
            wqk_slice(wqklv, wqkvl_d, 896, 640)
            # 4. V weights, bias rows, proj weights
            for w_t, w_d in ((wvhB, wqkvh_d), (wvlB, wqkvl_d)):
                nc.sync.dma_start(
                    out=w_t.rearrange("p (pp t f) -> p pp t f", t=2, f=E),
                    in_=w_d[0:768, 1536:].rearrange(
                        "(pp t k) f -> k pp t f", t=2, k=128))
            for w_t, w_d in ((wphB, wprojh_d), (wplB, wprojl_d)):
                nc.sync.dma_start(
                    out=w_t.rearrange("p (pp t f) -> p pp t f", t=2, f=E),
                    in_=w_d[0:512, :].rearrange(
                        "(pp t k) f -> k pp t f", t=2, k=128))
            nc.sync.dma_start(
                out=wpb45.rearrange("p (c f) -> p c f", f=E),
                in_=wpb45_d[:].rearrange("(c k) f -> k c f", k=128))

            def emit_prelude():
                for nf, (f0, fw) in enumerate(NFS):
                    pbv = ps.tile([128, 512], F32, name=f"pbv{nf}", tag="mm",
                                  bufs=2)
                    nc.tensor.matmul(pbv[:, :fw], ones32k,
                                     bv_row[:, f0:f0 + fw],
                                     start=True, stop=True)
                    nc.vector.tensor_copy(bv_bc[:, f0:f0 + fw], pbv[:, :fw])
                    pbp = ps.tile([128, 512], F32, name=f"pbp{nf}", tag="mm",
                                  bufs=2)
                    nc.tensor.matmul(pbp[:, :fw], ones32k,
                                     bp_row[:, f0:f0 + fw],
                                     start=True, stop=True)
                    nc.vector.tensor_copy(bp_bc[:, f0:f0 + fw], pbp[:, :fw])
                for i in range(NT):
                    nc.vector.memset(
                        vS[i].rearrange("p (h c) -> p h c", c=65)[:, :, 64:65],
                        1024.0)

            # ================= emission units =================
            def emit_qk_unit(t, q, evac_act=False):
                """One (feature-tile, 512-token-half) of Q or K projection.
                Split-fp8 DoubleRow: (xh+xl)@(wh+wl), xl@wl dropped."""
                dst = qT[t] if t < 6 else kT[t - 6]
                wcol0 = t * 128
                pq = ps.tile([128, 512], F32, name=f"pq{t}_{q}", tag="mm",
                             bufs=2)
                terms = [(wqkhv, xHv), (wqkhv, xLv), (wqklv, xHv)]
                nmm = 3 * len(terms)
                mi = 0
                for p in range(3):
                    for w_v, x_v in terms:
                        nc.tensor.matmul(
                            pq,
                            w_v[:, p, :, wcol0:wcol0 + 128],
                            x_v[:, 2 * p:2 * p + 2, q * 512:(q + 1) * 512],
                            start=(mi == 0), stop=(mi == nmm - 1),
                            perf_mode=DR)
                        mi += 1
                if evac_act:
                    nc.scalar.activation(
                        dst[:, q * 512:(q + 1) * 512], pq,
                        mybir.ActivationFunctionType.Identity,
                        bias=bqc[:, t:t + 1], scale=float(2.0 ** -15))
                else:
                    nc.vector.tensor_scalar(
                        out=dst[:, q * 512:(q + 1) * 512], in0=pq,
                        scalar1=float(2.0 ** -15), scalar2=bqc[:, t:t + 1],
                        op0=MUL, op1=ADD)

            def emit_v_unit(i, nf):
                """One (token-chunk, free-half) of the V projection."""
                f0, fw = NFS[nf]
                pv = ps.tile([128, 512], F32, name=f"pv{i}_{nf}", tag="mm",
                             bufs=2)
                terms = [(xHv, wvhv), (xHv, wvlv), (xLv, wvhv)]
                nmm = 3 * len(terms)
                mi = 0
                for p in range(3):
                    for x_v, w_v in terms:
                        nc.tensor.matmul(
                            pv[:, :fw],
                            x_v[:, 2 * p:2 * p + 2, i * 128:(i + 1) * 128],
                            w_v[:, p, :, f0:f0 + fw],
                            start=(mi == 0), stop=(mi == nmm - 1),
                            perf_mode=DR)
                        mi += 1
                nh, h0 = fw // D, f0 // D
                nc.vector.tensor_add(
                    vS[i].rearrange("p (h c) -> p h c", c=65)
                        [:, h0:h0 + nh, 0:64],
                    pv[:, :fw].rearrange("p (h d) -> p h d", d=D),
                    bv_bc[:, f0:f0 + fw].rearrange("p (h d) -> p h d", d=D))

            expS_of = {}

            def emit_s_unit(h, kc):
                """S^T[k-chunk, all q] for one head + exp -> bf16 expS."""
                c, r0 = h // 2, (h % 2) * 64
                if kc == 0:
                    expS_of[h] = [
                        ep.tile([128, N], BF16, name=f"eS{h}_{k2}",
                                tag="expS", bufs=24)
                        for k2 in range(NT)]
                pss = ps.tile([128, N], F32, name=f"ps{h}_{kc}", tag="s2",
                              bufs=2)
                for q in range(NQ):
                    nc.tensor.matmul(
                        pss[:, q * 512:(q + 1) * 512],
                        kT[c][r0:r0 + 64, kc * 128:(kc + 1) * 128],
                        qT[c][r0:r0 + 64, q * 512:(q + 1) * 512],
                        start=True, stop=True)
                nc.scalar.activation(expS_of[h][kc], pss, EXP,
                                     scale=float(SCALE))

            def emit_u_unit(h, qc, evac_act=False):
                """U^T[q-chunk, 65] for one head; normalize into attnS."""
                expS = expS_of[h]
                pu = ps.tile([128, 512], F32, name=f"pu{h}_{qc}", tag="u",
                             bufs=2)
                for kc in range(NT):
                    nc.tensor.matmul(
                        pu[:, 0:65],
                        expS[kc][:, qc * 128:(qc + 1) * 128],
                        vS[kc][:, h * 65:h * 65 + 65],
                        start=(kc == 0), stop=(kc == NT - 1))
                invz = iz.tile([128, 1], F32, name=f"iv{h}_{qc}", tag="iz",
                               bufs=3)
                nc.vector.reciprocal(invz, pu[:, 64:65])
                if evac_act:
                    nc.scalar.activation(
                        attnS[qc][:, h * D:(h + 1) * D], pu[:, 0:64],
                        mybir.ActivationFunctionType.Copy, scale=invz)
                else:
                    nc.vector.tensor_scalar_mul(
                        attnS[qc][:, h * D:(h + 1) * D], pu[:, 0:64], invz)

            def emit_transp(qc, pp):
                """DMA-transpose one block pair into attnT (32*attn^T bf16),
                then split to fp8 hi/lo on the Pool engine."""
                c0 = 2 * pp
                t_bf = attnTv[:, c0:c0 + 2, qc * 128:(qc + 1) * 128]
                nc.sync.dma_start_transpose(
                    t_bf, attnS[qc][:, c0 * 128:(c0 + 2) * 128])
                if pp < 2:
                    t_hi = attnThv[:, c0:c0 + 2, qc * 128:(qc + 1) * 128]
                    nc.gpsimd.tensor_copy(t_hi, t_bf)
                    nc.gpsimd.tensor_sub(
                        attnTlv[:, c0:c0 + 2, qc * 128:(qc + 1) * 128],
                        t_bf, t_hi)

            def emit_proj_unit(i, nf, pp):
                """Half-row of output proj for block pair pp (split-fp8 DR).
                pp=0 adds the (2^15-scaled) bias; pp=1 accumulates; pp=2
                accumulates and rescales to the final f32 output."""
                f0, fw = NFS[nf]
                po = ps.tile([128, 512], F32, name=f"po{i}_{nf}_{pp}",
                             tag="mm", bufs=2)
                if pp < 2:
                    terms = [(attnThv, wphv), (attnThv, wplv), (attnTlv, wphv)]
                    for mi, (a_v, w_v) in enumerate(terms):
                        nc.tensor.matmul(
                            po[:, :fw],
                            a_v[:, 2 * pp:2 * pp + 2, i * 128:(i + 1) * 128],
                            w_v[:, pp, :, f0:f0 + fw],
                            start=(mi == 0), stop=(mi == len(terms) - 1),
                            perf_mode=DR)
                else:
                    for ci, cb in enumerate((4, 5)):
                        nc.tensor.matmul(
                            po[:, :fw],
                            attnTv[:, cb, i * 128:(i + 1) * 128],
                            wpb45v[:, cb - 4, f0:f0 + fw],
                            start=(ci == 0), stop=(ci == 1))
                if pp == 0:
                    nc.vector.tensor_add(
                        o_acc[i][:, f0:f0 + fw], po[:, :fw],
                        bp_bc[:, f0:f0 + fw])
                elif pp == 1:
                    nc.vector.tensor_add(
                        o_acc[i][:, f0:f0 + fw], po[:, :fw],
                        o_acc[i][:, f0:f0 + fw])
                    # rescale the pairs-0/1(+bias) partial here, off the tail
                    nc.gpsimd.tensor_scalar_mul(
                        o_acc[i][:, f0:f0 + fw], o_acc[i][:, f0:f0 + fw],
                        float(2.0 ** -15))
                else:
                    nc.vector.tensor_add(
                        o_acc[i][:, f0:f0 + fw], po[:, :fw],
                        o_acc[i][:, f0:f0 + fw])

            # ================= schedule =================
            for q in range(NQ):
                for t in (0, 6):
                    emit_qk_unit(t, q)

            for h in range(H):
                c = h // 2
                fillers = []
                if h == 0:
                    fillers.append(("prelude",))
                if h < 10:  # QK chunk c+1: 2 units during each of h=2c, 2c+1
                    t0 = (c + 1, 6 + c + 1)
                    if h % 2 == 0:
                        fillers.append(("qk", t0[0], 0))
                        fillers.append(("qk", t0[1], 0))
                    else:
                        fillers.append(("qk", t0[0], 1))
                        fillers.append(("qk", t0[1], 1))
                if h in (1, 2):  # V projection (wv lands early now)
                    lo, hi = (0, 3) if h == 1 else (3, 8)
                    for i in range(lo, hi):
                        fillers.append(("v", i, 0))
                        fillers.append(("v", i, 1))
                if h >= 2:  # U for head h-2 (after V units at h=2)
                    for qc in range(NT):
                        fillers.append(("u", h - 2, qc))
                if h == 10:
                    fillers.append(("xfree",))
                if h in (6, 7):  # proj pair 0 (blocks 0,1; ready after U(3))
                    for i in range(4 * (h - 6), 4 * (h - 6) + 4):
                        fillers.append(("pa", i, 0, 0))
                        fillers.append(("pa", i, 1, 0))
                if h in (10, 11):  # proj pair 1 (blocks 2,3; after U(7))
                    for i in range(4 * (h - 10), 4 * (h - 10) + 4):
                        fillers.append(("pa", i, 0, 1))
                        fillers.append(("pa", i, 1, 1))
                if h == 11:  # U(10) late in head 11 (exp(10) done by then)
                    for qc in range(NT):
                        fillers.append(("u", 10, qc))

                fillers.sort(key=lambda f: f[0] == "u")

                fillers.sort(key=lambda f: f[0] == "u")

                def drain(k):
                    for _ in range(k):
                        if not fillers:
                            return
                        f = fillers.pop(0)
                        if f[0] == "v":
                            emit_v_unit(f[1], f[2])
                        elif f[0] == "qk":
                            emit_qk_unit(f[1], f[2])
                        elif f[0] == "u":
                            emit_u_unit(f[1], f[2])
                            if f[1] == 3:
                                emit_transp(f[2], 0)
                            elif f[1] == 7:
                                emit_transp(f[2], 1)
                        elif f[0] == "pa":
                            emit_proj_unit(f[1], f[2], f[3])
                        elif f[0] == "xfree":
                            xp.release()
                        elif f[0] == "prelude":
                            emit_prelude()

                per = (len(fillers) + NT - 1) // NT
                for kc in range(NT):
                    emit_s_unit(h, kc)
                    drain(per)
                drain(len(fillers))

            # ---- tail: U(11) + attnT blocks 4,5 + proj pair 2, pipelined
            def emit_out(i):
                emit_proj_unit(i, 0, 2)
                emit_proj_unit(i, 1, 2)
                nc.sync.dma_start(
                    out=out_d[i * 128:(i + 1) * 128, :], in_=o_acc[i])

            for qc in range(NT):
                emit_u_unit(11, qc)
                emit_transp(qc, 2)
                if qc >= 1:
                    emit_out(qc - 1)
            emit_out(NT - 1)
            iz.release()
            ep.release()
    nc.compile()
    return nc


_NC_CACHE = None


def kernel(x, W_qkv, b_qkv, W_proj, b_proj):
    from concourse.bass_utils import run_bass_kernel_spmd
    import ml_dtypes

    F8NP = ml_dtypes.float8_e4m3
    BF16NP = ml_dtypes.bfloat16

    global _NC_CACHE
    if _NC_CACHE is None:
        _NC_CACHE = _build()
    nc = _NC_CACHE

    x = np.asarray(x, dtype=np.float32)
    W_qkv = np.asarray(W_qkv, dtype=np.float32)
    b_qkv = np.ascontiguousarray(np.asarray(b_qkv, dtype=np.float32))
    W_proj = np.asarray(W_proj, dtype=np.float32)
    b_proj = np.asarray(b_proj, dtype=np.float32)

    # host-side preprocessing: transposed split-fp8 x, split-fp8 W_qkv,
    # bf16 W_proj and bias rows
    def split8(a):
        hi = a.astype(F8NP)
        lo = (a - hi.astype(np.float32)).astype(F8NP)
        return np.ascontiguousarray(hi), np.ascontiguousarray(lo)

    w_h, w_l = split8(W_qkv * np.float32(1024.0))
    xt32 = np.ascontiguousarray(np.swapaxes(x, 1, 2)) * np.float32(32.0)
    xsplit = [split8(xt32[b]) for b in range(B)]
    wp_h, wp_l = split8(W_proj * np.float32(1024.0))
    wp45 = np.ascontiguousarray(
        (W_proj[512:, :] * np.float32(1.0 / 32.0)).astype(BF16NP))
    bv_b = np.ascontiguousarray(b_qkv[2 * E:].astype(BF16NP))
    bp_b = np.ascontiguousarray(b_proj.astype(BF16NP))

    in_maps = [
        {"xh": xsplit[b][0], "xl": xsplit[b][1],
         "W_qkvh": w_h, "W_qkvl": w_l, "b_qkv": b_qkv,
         "b_v": bv_b, "W_projh": wp_h, "W_projl": wp_l, "W_pb45": wp45,
         "b_pb": bp_b}
        for b in range(B)
    ]
    res = run_bass_kernel_spmd(nc, in_maps, core_ids=list(range(B)))
    return np.stack([np.asarray(res.results[b]["out"]) for b in range(B)])
